# revision 15
# baseline (speedup 1.0000x reference)
"""Trainium2 Bass kernel for an MQA attention block (8 q-heads, shared K/V).

Sharding: 8 cores; core c -> batch b=c//4, query rows s0=(c%4)*512 .. +512,
all 8 heads.  K/V (full sequence, per batch) computed redundantly per core.

v3 design notes (vs v2 baseline):
- Fully fused single pipeline: per-head Q-projection/LN/rope, QK, softcap
  (tanh+exp on ACT), and PV are software-pipelined so the ACT engine (the
  hard bottleneck: 2 table passes over S*S*H logits) never waits.
- All projections in bf16 (x, Wq, Wk, Wv streamed bf16; halves DMA traffic,
  enables FWL weight loads).  K/V/Q projections are "row-direct": the x
  chunk is the stationary operand so k/v/q come out row-major -- no
  forward transposes and no ACT drain copies.
- Attention bias added via DVE tensor_tensor into the QK PSUM (frees the PE
  from the identity-matmul bias preload of v2).
- q-LN rstd via Newton rsqrt on GPSIMD (avoids ACT Sqrt table switches in
  the tanh/exp steady state); rope also on GPSIMD.
- ACT does ONLY tanh+exp in the head loop.
"""

import os
import sys

for _p in ("/opt/trn_rl_repo",):
    if _p not in sys.path and os.path.isdir(_p):
        sys.path.insert(0, _p)

import numpy as np
from contextlib import ExitStack

import concourse.bass as bass
import concourse.mybir as mybir
import concourse.tile as tile
from concourse import bacc
from concourse import bass_utils

F32 = mybir.dt.float32
BF16 = mybir.dt.bfloat16

B, S, D = 2, 2048, 1536
H, DQ, DK, DV = 8, 128, 128, 192
P = 128
SQ = S // 4          # 512 query rows per core
DC = D // P          # 12 contraction chunks
JC = S // P          # 16 key chunks
SC = SQ // P         # 4 query-row chunks
NCORES = 8
EPS_RMS = 1e-6
EPS_LN = 1e-5
SOFTCAP = 5.0
ROPE_BASE = 8192.0
HALF = DQ // 2
VW = 256             # vrow inner stride; cols 0:192 v, 192 ones

TT = mybir.AluOpType
AF = mybir.ActivationFunctionType


def build_program(has_rbq=False, has_rbk=False, has_b=False):
    nc = bacc.Bacc(
        "TRN2", target_bir_lowering=False, debug=False, num_devices=NCORES
    )

    def din(name, shape, dt=BF16):
        return nc.dram_tensor(name, list(shape), dt, kind="ExternalInput").ap()

    # x in device layout [P, JC, DC, P]: [p, jc, dc, col] = x.T[dc*P+p, jc*P+col]
    xT = din("xT", (P, JC, DC, P))
    xq = din("xq", (P, DC, SQ))          # per-core query-column slice of x.T
    biasT = din("biasT", (P, JC, SQ))
    c1q_t = din("c1q", (P, SC, HALF))
    s2nq_t = din("s2nq", (P, SC, HALF))
    s1q_t = din("s1q", (P, SC, HALF))
    c2q_t = din("c2q", (P, SC, HALF))
    c1k_t = din("c1k", (P, JC, HALF))
    s2nk_t = din("s2nk", (P, JC, HALF))
    s1k_t = din("s1k", (P, JC, HALF))
    c2k_t = din("c2k", (P, JC, HALF))
    wq_t = din("wq", (P, DC, H * DQ))
    wk_t = din("wk", (P, DC, DK))
    wv_t = din("wv", (P, DC, DV))
    wo_t = din("wo", (P, DC, D))
    bor_t = din("bor", (P, D), F32)
    identb_t = din("identb", (P, P))
    if has_b:
        # packed row biases: [bk (DK), bv (DV), bq (H*DQ)]
        brow_t = din("brow", (1, DK + DV + H * DQ))
    if has_rbq:
        rbq_t = din("rbq", (P, SC, DQ))
    if has_rbk:
        rbk_t = din("rbk", (P, JC, DK))
    out = nc.dram_tensor("out", [SQ, D], F32, kind="ExternalOutput").ap()
    DEBUG = os.environ.get("KDEBUG", "0") == "1"
    if DEBUG:
        dbg_kT = nc.dram_tensor("dbg_kT", [P, S], BF16, kind="ExternalOutput").ap()
        dbg_vrow = nc.dram_tensor("dbg_vrow", [P, JC, VW], BF16, kind="ExternalOutput").ap()
        dbg_qT0 = nc.dram_tensor("dbg_qT0", [P, SQ], BF16, kind="ExternalOutput").ap()
        dbg_pts0 = nc.dram_tensor("dbg_pts0", [P, JC, SQ], BF16, kind="ExternalOutput").ap()
        dbg_yp0 = nc.dram_tensor("dbg_yp0", [P, SC, 2 * DV], BF16, kind="ExternalOutput").ap()

    with tile.TileContext(nc) as tc, ExitStack() as ctx:
        const = ctx.enter_context(tc.tile_pool(name="const", bufs=1))
        persist = ctx.enter_context(tc.tile_pool(name="persist", bufs=1))
        qt = ctx.enter_context(tc.tile_pool(name="qt", bufs=2))
        qps_cm = tc.tile_pool(name="qps", bufs=1, space="PSUM")
        qpsp = qps_cm.__enter__()
        scr_cm = tc.tile_pool(name="scr", bufs=2, space="PSUM")
        scr = scr_cm.__enter__()

        # ---------------- constants ----------------
        identb = const.tile([P, P], BF16)
        nc.sync.dma_start(identb[:], identb_t)
        eps_sb = const.tile([P, 1], F32)
        nc.vector.memset(eps_sb[:], EPS_LN)
        if has_b:
            brow = const.tile([1, DK + DV + H * DQ], BF16)
            nc.sync.dma_start(brow[:], brow_t)
            ones1 = const.tile([1, P], BF16)
            nc.vector.memset(ones1[:], 1.0)

        wk_sb = const.tile([P, DC, DK], BF16)
        nc.sync.dma_start(wk_sb[:], wk_t)
        wv_sb = const.tile([P, DC, DV], BF16)
        nc.sync.dma_start(wv_sb[:], wv_t)

        def load_tab(t, n, nm):
            tt = const.tile([P, n, HALF], BF16, tag=nm, name=nm)
            nc.sync.dma_start(tt[:], t)
            return tt

        c1k = load_tab(c1k_t, JC, "c1k")
        s2nk = load_tab(s2nk_t, JC, "s2nk")
        s1k = load_tab(s1k_t, JC, "s1k")
        c2k = load_tab(c2k_t, JC, "c2k")
        if has_rbk:
            rbk = const.tile([P, JC, DK], BF16)
            nc.sync.dma_start(rbk[:], rbk_t)
        xq_sb = persist.tile([P, DC, SQ], BF16)
        nc.sync.dma_start(xq_sb[:], xq)
        c1q = load_tab(c1q_t, SC, "c1q")
        s2nq = load_tab(s2nq_t, SC, "s2nq")
        s1q = load_tab(s1q_t, SC, "s1q")
        c2q = load_tab(c2q_t, SC, "c2q")
        if has_rbq:
            rbq = const.tile([P, SC, DQ], BF16)
            nc.sync.dma_start(rbq[:], rbq_t)

        # ---------------- persistent activations ----------------
        kT_sb = persist.tile([P, S], BF16)          # rope'd k, [dk, s]
        vrow_sb = persist.tile([P, JC, VW], BF16)   # v rows + ones col
        nc.vector.memset(vrow_sb[:, :, DV : DV + 1], 1.0)
        qT = [
            persist.tile([P, SQ], BF16, tag=f"q{h}", name=f"qT{h}")
            for h in range(H)
        ]
        yp = [
            persist.tile([P, SC, 2 * DV], BF16, tag=f"yp{p}", name=f"yp{p}")
            for p in range(4)
        ]
        yT_sb = persist.tile([P, DC, SQ], BF16)

        g = nc.gpsimd if os.environ.get("USE_GPSIMD", "1") == "1" else nc.vector

        # =========================================================
        # KV phase: row-direct projections, per key chunk
        # =========================================================
        with (
            tc.tile_pool(name="kvx", bufs=3) as kvx,
            tc.tile_pool(name="kvt", bufs=2) as kvt,
            tc.tile_pool(name="kvps", bufs=2, space="PSUM") as kvps,
        ):
            for jc in range(JC):
                xt = kvx.tile([P, DC, P], BF16, tag="xt", name=f"xt{jc}")
                nc.sync.dma_start(xt[:], xT[:, jc, :, :])
                k_ps = kvps.tile([P, DK], F32, tag="k_ps",
                                 name=f"kps{jc}")
                v_ps = kvps.tile([P, DV], F32, tag="v_ps",
                                 name=f"vps{jc}")
                last = DC - 1
                for dc in range(DC):
                    st = dc == 0
                    sp = (dc == last) and not has_b
                    nc.tensor.matmul(
                        k_ps, xt[:, dc, :], wk_sb[:, dc, :],
                        start=st, stop=sp,
                    )
                    nc.tensor.matmul(
                        v_ps, xt[:, dc, :], wv_sb[:, dc, :],
                        start=st, stop=sp,
                    )
                if has_b:
                    nc.tensor.matmul(
                        k_ps, ones1[:], brow[:, 0:DK],
                        start=False, stop=True,
                    )
                    nc.tensor.matmul(
                        v_ps, ones1[:], brow[:, DK : DK + DV],
                        start=False, stop=True,
                    )

                # stats directly on PSUM rows
                kst = kvt.tile([P, 6], F32, tag="kst", name=f"kst{jc}")
                kag = kvt.tile([P, 2], F32, tag="kag", name=f"kag{jc}")
                nc.vector.bn_stats(kst[:], k_ps)
                nc.vector.bn_aggr(kag[:], kst[:])
                vst = kvt.tile([P, 6], F32, tag="vst", name=f"vst{jc}")
                vag = kvt.tile([P, 2], F32, tag="vag", name=f"vag{jc}")
                nc.vector.bn_stats(vst[:], v_ps)
                nc.vector.bn_aggr(vag[:], vst[:])
                rst = kvt.tile([P, 2], F32, tag="rst", name=f"rst{jc}")
                nc.scalar.activation(
                    rst[:, 0:1], kag[:, 1:2], AF.Sqrt, bias=eps_sb[:, 0:1]
                )
                nc.scalar.activation(
                    rst[:, 1:2], vag[:, 1:2], AF.Sqrt, bias=eps_sb[:, 0:1]
                )
                nc.vector.reciprocal(rst[:], rst[:])

                # normalize k -> rows, v -> vrow_sb directly
                xnk = kvt.tile([P, DK], BF16, tag="xnk", name=f"xnk{jc}")
                nc.vector.tensor_scalar(
                    xnk[:], k_ps, kag[:, 0:1], rst[:, 0:1],
                    TT.subtract, TT.mult,
                )
                nc.vector.tensor_scalar(
                    vrow_sb[:, jc, 0:DV], v_ps, vag[:, 0:1], rst[:, 1:2],
                    TT.subtract, TT.mult,
                )

                # rope k on gpsimd
                kz1 = kvt.tile([P, HALF], BF16, tag="kz1", name=f"kz1{jc}")
                kz2 = kvt.tile([P, HALF], BF16, tag="kz2", name=f"kz2{jc}")
                kr = kvt.tile([P, DK], BF16, tag="kr", name=f"kr{jc}")
                g.tensor_tensor(kz1[:], xnk[:, :HALF], c1k[:, jc, :], TT.mult)
                g.tensor_tensor(kz2[:], xnk[:, HALF:], s2nk[:, jc, :], TT.mult)
                g.tensor_tensor(kr[:, :HALF], kz1[:], kz2[:], TT.add)
                g.tensor_tensor(kz1[:], xnk[:, :HALF], s1k[:, jc, :], TT.mult)
                g.tensor_tensor(kz2[:], xnk[:, HALF:], c2k[:, jc, :], TT.mult)
                g.tensor_tensor(kr[:, HALF:], kz1[:], kz2[:], TT.add)
                if has_rbk:
                    g.tensor_tensor(kr[:], kr[:], rbk[:, jc, :], TT.add)

                # back transpose into kT
                scb = scr.tile([P, P], BF16, tag="scb", name=f"kbt{jc}")
                nc.tensor.transpose(scb[:], kr[:], identb[:])
                nc.vector.tensor_copy(kT_sb[:, jc * P : (jc + 1) * P], scb[:])

                if jc == 2:
                    wq_sb = persist.tile([P, DC, H * DQ], BF16)
                    nc.sync.dma_start(wq_sb[:], wq_t)
                if jc == 6:
                    biasT_sb = persist.tile([P, JC, SQ], BF16)
                    nc.sync.dma_start(biasT_sb[:], biasT)
                if jc == 10:
                    wo_sb = persist.tile([P, DC, D], BF16)
                    nc.sync.dma_start(wo_sb[:], wo_t)
                    bor = persist.tile([P, D], F32)
                    nc.sync.dma_start(bor[:], bor_t)

        # =========================================================
        # per-head q pipeline (row-direct)
        # =========================================================
        def stage_q_proj(h, t, parts):
            """project q chunk t of head h into the head's shared PSUM tile"""
            if t == 0:
                parts.append(
                    qpsp.tile([P, SC, DQ], F32, tag="q_ps", name=f"qps{h}")
                )
            q_ps = parts[0]
            last = DC - 1
            for dc in range(DC):
                nc.tensor.matmul(
                    q_ps[:, t, :],
                    xq_sb[:, dc, t * P : (t + 1) * P],
                    wq_sb[:, dc, h * DQ : (h + 1) * DQ],
                    start=(dc == 0), stop=(dc == last) and not has_b,
                )
            if has_b:
                nc.tensor.matmul(
                    q_ps[:, t, :], ones1[:],
                    brow[:, DK + DV + h * DQ : DK + DV + (h + 1) * DQ],
                    start=False, stop=True,
                )
            st6 = qt.tile([P, 6], F32, tag=f"qst{t}", name=f"qst{h}_{t}")
            nc.vector.bn_stats(st6[:], q_ps[:, t, :])
            parts.append(st6)

        def stage_q_ln(h, parts):
            """newton rsqrt + normalize + rope + transpose into qT[h]"""
            q_ps = parts[0]
            qag = qt.tile([P, SC, 2], F32, tag="qag", name=f"qag{h}")
            for t in range(SC):
                nc.vector.bn_aggr(qag[:, t, :], parts[1 + t][:])
            # newton rsqrt on gpsimd: y = rsqrt(var + eps)
            qv = qt.tile([P, SC, 1], F32, tag="qv", name=f"qv{h}")
            qy = qt.tile([P, SC, 1], F32, tag="qy", name=f"qy{h}")
            qw2 = qt.tile([P, SC, 1], F32, tag="qw2", name=f"qw2{h}")
            g.tensor_scalar(qv[:], qag[:, :, 1:2], EPS_LN, None, TT.add)
            g.tensor_scalar(qy[:], qv[:], -0.5, 1.5, TT.mult, TT.add)
            for _ in range(3):
                g.tensor_tensor(qw2[:], qy[:], qy[:], TT.mult)
                g.tensor_tensor(qw2[:], qw2[:], qv[:], TT.mult)
                g.tensor_scalar(qw2[:], qw2[:], -0.5, 1.5, TT.mult, TT.add)
                g.tensor_tensor(qy[:], qy[:], qw2[:], TT.mult)
            xnq = qt.tile([P, SC, DQ], BF16, tag="xnq", name=f"xnq{h}")
            for t in range(SC):
                nc.vector.tensor_scalar(
                    xnq[:, t, :], q_ps[:, t, :],
                    qag[:, t, 0:1], qy[:, t, 0:1],
                    TT.subtract, TT.mult,
                )
            qz1 = qt.tile([P, SC, HALF], BF16, tag="qz1", name=f"qz1{h}")
            qz2 = qt.tile([P, SC, HALF], BF16, tag="qz2", name=f"qz2{h}")
            qr = qt.tile([P, SC, DQ], BF16, tag="qr", name=f"qr{h}")
            g.tensor_tensor(qz1[:], xnq[:, :, :HALF], c1q[:], TT.mult)
            g.tensor_tensor(qz2[:], xnq[:, :, HALF:], s2nq[:], TT.mult)
            g.tensor_tensor(qr[:, :, :HALF], qz1[:], qz2[:], TT.add)
            g.tensor_tensor(qz1[:], xnq[:, :, :HALF], s1q[:], TT.mult)
            g.tensor_tensor(qz2[:], xnq[:, :, HALF:], c2q[:], TT.mult)
            g.tensor_tensor(qr[:, :, HALF:], qz1[:], qz2[:], TT.add)
            if has_rbq:
                g.tensor_tensor(qr[:], qr[:], rbq[:], TT.add)
            for t in range(SC):
                sct = scr.tile([P, P], BF16, tag="scb", name=f"qbt{h}{t}")
                nc.tensor.transpose(sct[:], qr[:, t, :], identb[:])
                nc.vector.tensor_copy(qT[h][:, t * P : (t + 1) * P], sct[:])

        # emit q pipeline for heads 0 and 1 up front
        parts0 = []
        for t in range(SC):
            stage_q_proj(0, t, parts0)
        stage_q_ln(0, parts0)
        parts1 = []
        for t in range(SC):
            stage_q_proj(1, t, parts1)
        stage_q_ln(1, parts1)

        # =========================================================
        # head loop: QK + softcap + PV pipelined; stageQ(h+2) and
        # PV(h-1) interleaved as PE fillers
        # =========================================================
        with (
            tc.tile_pool(name="att", bufs=2) as att,
            tc.tile_pool(name="apq", bufs=2, space="PSUM") as apq,
            tc.tile_pool(name="ay", bufs=1, space="PSUM") as ay,
        ):
            pts = {}

            def qk_group(h, jg):
                pq = apq.tile([P, 2, SQ], F32, tag="pq", name=f"pq{h}{jg}")
                for c in range(2):
                    jc = jg * 2 + c
                    nc.tensor.matmul(
                        pq[:, c, :],
                        kT_sb[:, jc * P : (jc + 1) * P], qT[h][:],
                        start=True, stop=True,
                    )
                nc.vector.tensor_tensor(
                    pq[:], pq[:], biasT_sb[:, jg * 2 : jg * 2 + 2, :], TT.add
                )
                nc.scalar.activation(
                    pq[:], pq[:], AF.Tanh, scale=1.0 / SOFTCAP
                )
                nc.scalar.activation(
                    pts[h][:, jg * 2 : jg * 2 + 2, :], pq[:],
                    AF.Exp, scale=SOFTCAP,
                )

            def pv_chunk(h, ic):
                y_ps = ay.tile([P, DV + 1], F32, tag="y_ps",
                               name=f"yps{h}{ic}")
                for jc in range(JC):
                    nc.tensor.matmul(
                        y_ps[:],
                        pts[h][:, jc, ic * P : (ic + 1) * P],
                        vrow_sb[:, jc, : DV + 1],
                        start=(jc == 0), stop=(jc == JC - 1),
                    )
                rcp = att.tile([P, 1], F32, tag="rcp", name=f"rcp{h}{ic}")
                nc.vector.reciprocal(rcp[:], y_ps[:, DV : DV + 1])
                nc.vector.tensor_scalar_mul(
                    yp[h // 2][:, ic, (h % 2) * DV : (h % 2 + 1) * DV],
                    y_ps[:, :DV], rcp[:, 0:1],
                )

            def oproj_transposes(p):
                # yp[p] (heads 2p,2p+1) -> yT chunks 3p..3p+2
                for sc in range(SC):
                    for fcl in range(3):
                        fc = 3 * p + fcl
                        scb = scr.tile([P, P], BF16, tag="scb",
                                       name=f"so{p}{sc}{fcl}")
                        nc.tensor.transpose(
                            scb[:],
                            yp[p][:, sc, fcl * P : (fcl + 1) * P],
                            identb[:],
                        )
                        nc.vector.tensor_copy(
                            yT_sb[:, fc, sc * P : (sc + 1) * P], scb[:]
                        )

            for h in range(H):
                pts[h] = att.tile([P, JC, SQ], BF16, tag="pt", name=f"pt{h}")
                # build filler list for this head
                fillers = []
                if h + 2 < H:
                    hh = h + 2
                    partsn = []
                    for t in range(SC):
                        fillers.append(
                            lambda hh=hh, t=t, pn=partsn: stage_q_proj(
                                hh, t, pn
                            )
                        )
                    fillers.append(
                        lambda hh=hh, pn=partsn: stage_q_ln(hh, pn)
                    )
                if h > 0:
                    for ic in range(SC):
                        fillers.append(
                            lambda h=h, ic=ic: pv_chunk(h - 1, ic)
                        )
                if h >= 3 and h % 2 == 1:
                    # yp[(h-3)//2 .. ] done after pv(h-2) ran in head h-1
                    fillers.append(lambda p=(h - 3) // 2: oproj_transposes(p))

                nf = len(fillers)
                done = 0
                for jg in range(JC // 2):
                    qk_group(h, jg)
                    want = (nf * (jg + 1) + 7) // 8
                    while done < want:
                        fillers[done]()
                        done += 1
            # tail: PV of last head, remaining transposes
            for ic in range(SC):
                pv_chunk(H - 1, ic)
            oproj_transposes(3)
            if DEBUG:
                nc.sync.dma_start(dbg_kT[:], kT_sb[:])
                nc.sync.dma_start(dbg_vrow[:], vrow_sb[:])
                nc.sync.dma_start(dbg_qT0[:], qT[0][:])
                nc.sync.dma_start(dbg_pts0[:], pts[0][:])
                nc.sync.dma_start(dbg_yp0[:], yp[0][:])

        scr_cm.__exit__(None, None, None)
        qps_cm.__exit__(None, None, None)

        # =========================================================
        # Output projection (bf16)
        # =========================================================
        with (
            tc.tile_pool(name="od", bufs=2) as od,
            tc.tile_pool(name="ops", bufs=2, space="PSUM") as ops,
        ):
            for sc in range(SC):
                o_ps = ops.tile([P, D], F32, tag="o_ps", name=f"ops{sc}")
                for fc in range(DC):
                    for n in range(D // 512):
                        nc.tensor.matmul(
                            o_ps[:, n * 512 : (n + 1) * 512],
                            yT_sb[:, fc, sc * P : (sc + 1) * P],
                            wo_sb[:, fc, n * 512 : (n + 1) * 512],
                            start=(fc == 0), stop=(fc == DC - 1),
                        )
                o_sb = od.tile([P, D], F32, tag="o_sb", name=f"osb{sc}")
                nc.vector.tensor_tensor(o_sb[:], o_ps[:], bor[:], TT.add)
                nc.sync.dma_start(out[sc * P : (sc + 1) * P, :], o_sb[:])

    nc.compile()
    return nc


def _host_prep(inputs):
    import ml_dtypes

    f32 = np.float32
    bf16 = ml_dtypes.bfloat16
    x = np.asarray(inputs["x"], f32)
    bias = np.asarray(inputs["attention_bias"], f32)
    g1 = np.asarray(inputs["g1"], f32)
    b1 = np.asarray(inputs["b1"], f32)
    rr1 = np.asarray(inputs["rrms1"], f32)
    Wq = np.asarray(inputs["Wq"], f32)
    Wk = np.asarray(inputs["Wk"], f32)
    Wv = np.asarray(inputs["Wv"], f32)
    qg = np.asarray(inputs["qg"], f32)
    qb = np.asarray(inputs["qb"], f32)
    kg = np.asarray(inputs["kg"], f32)
    kb = np.asarray(inputs["kb"], f32)
    vg = np.asarray(inputs["vg"], f32)
    vb = np.asarray(inputs["vb"], f32)
    Wo = np.asarray(inputs["Wo"], f32)
    bo = np.asarray(inputs["bo"], f32)
    g2 = np.asarray(inputs["g2"], f32)
    b2 = np.asarray(inputs["b2"], f32)
    rr2 = np.asarray(inputs["rrms2"], f32)

    scale1 = (g1 * (1.0 / np.sqrt(rr1 + EPS_RMS))).astype(f32)
    Wq_e = (Wq * scale1[:, None]).astype(f32)
    Wk_e = (Wk * scale1[:, None]).astype(f32)
    Wv_e = (Wv * scale1[:, None]).astype(f32)
    bq_row = (b1 @ Wq).astype(f32)      # [H*DQ]
    bk_row = (b1 @ Wk).astype(f32)      # [DK]
    bv_row = (b1 @ Wv).astype(f32)      # [DV]
    sc_q = f32(DQ) ** f32(-0.5)
    qg_e = (qg * sc_q).astype(f32)
    qb_e = (qb * sc_q).astype(f32)

    # v-affine folded through attention into Wo/bo; rms2 folded too
    scale2 = (g2 * (1.0 / np.sqrt(rr2 + EPS_RMS))).astype(f32)
    vg_rep = np.tile(vg, H)                      # [H*DV]
    Wo_e = (Wo * vg_rep[:, None] * scale2[None, :]).astype(f32)
    vb_fold = (np.tile(vb, H) @ Wo).astype(f32)  # [D]
    bo_e = ((bo + vb_fold) * scale2 + b2).astype(f32)

    freqs = (
        1.0 / (ROPE_BASE ** (np.arange(HALF, dtype=f32) / HALF))
    ).astype(f32)
    ang = np.arange(S, dtype=f32)[:, None] * freqs[None, :]
    cos = np.cos(ang).astype(f32)                        # [S, 64]
    sin = np.sin(ang).astype(f32)

    # rope tables with gamma folded (and DQ^-0.5 for q)
    c1k = (cos * kg[None, :HALF]).astype(bf16)
    s2nk = (-sin * kg[None, HALF:]).astype(bf16)
    s1k = (sin * kg[None, :HALF]).astype(bf16)
    c2k = (cos * kg[None, HALF:]).astype(bf16)

    # rope'd beta tables (rope(b) by position)
    rbk_f = np.concatenate(
        [cos * kb[None, :HALF] - sin * kb[None, HALF:],
         sin * kb[None, :HALF] + cos * kb[None, HALF:]], axis=1
    ).astype(f32)
    rbq_f = np.concatenate(
        [cos * qb_e[None, :HALF] - sin * qb_e[None, HALF:],
         sin * qb_e[None, :HALF] + cos * qb_e[None, HALF:]], axis=1
    ).astype(f32)
    has_rbk = bool(np.any(rbk_f))
    has_rbq = bool(np.any(rbq_f))
    has_b = bool(np.any(bq_row) or np.any(bk_row) or np.any(bv_row))

    def dev3(a, n):
        """[n*P, W] row-major -> [P, n, W] device layout, contiguous."""
        return np.ascontiguousarray(
            a.reshape(n, P, a.shape[-1]).transpose(1, 0, 2)
        )

    rep = lambda v: np.ascontiguousarray(
        np.broadcast_to(v[None, :], (P, v.shape[0]))
    )
    shared = {
        "c1k": dev3(c1k, JC),
        "s2nk": dev3(s2nk, JC),
        "s1k": dev3(s1k, JC),
        "c2k": dev3(c2k, JC),
        "wq": dev3(Wq_e.astype(bf16), DC),
        "wk": dev3(Wk_e.astype(bf16), DC),
        "wv": dev3(Wv_e.astype(bf16), DC),
        "wo": dev3(Wo_e.astype(bf16), DC),
        "bor": rep(bo_e),
        "identb": np.eye(P, dtype=bf16),
    }
    if has_b:
        shared["brow"] = np.concatenate(
            [bk_row, bv_row, bq_row]
        ).astype(bf16)[None, :]
    if has_rbk:
        shared["rbk"] = dev3(rbk_f.astype(bf16), JC)

    # x device layout [P, JC, DC, P]
    xdev = []
    for b in range(B):
        xTb = np.ascontiguousarray(x[b].T).astype(bf16)      # [D, S]
        xdev.append(np.ascontiguousarray(
            xTb.reshape(DC, P, JC, P).transpose(1, 2, 0, 3)
        ))
    xTs = [np.ascontiguousarray(x[b].T) for b in range(B)]
    in_maps = []
    for c in range(NCORES):
        b = c // 4
        s0 = (c % 4) * SQ
        m = dict(shared)
        m["xT"] = xdev[b]
        m["xq"] = dev3(xTs[b][:, s0 : s0 + SQ].astype(bf16), DC)
        m["biasT"] = dev3(bias[0, 0, s0 : s0 + SQ, :].T.astype(bf16), JC)
        m["c1q"] = dev3(
            (cos[s0 : s0 + SQ] * qg_e[None, :HALF]).astype(bf16), SC
        )
        m["s2nq"] = dev3(
            (-sin[s0 : s0 + SQ] * qg_e[None, HALF:]).astype(bf16), SC
        )
        m["s1q"] = dev3(
            (sin[s0 : s0 + SQ] * qg_e[None, :HALF]).astype(bf16), SC
        )
        m["c2q"] = dev3(
            (cos[s0 : s0 + SQ] * qg_e[None, HALF:]).astype(bf16), SC
        )
        if has_rbq:
            m["rbq"] = dev3(rbq_f[s0 : s0 + SQ].astype(bf16), SC)
        in_maps.append(m)
    return in_maps, has_rbq, has_rbk, has_b


_NC_CACHE = {}


def _get_nc(has_rbq=False, has_rbk=False, has_b=False):
    key = (has_rbq, has_rbk, has_b)
    if key not in _NC_CACHE:
        _NC_CACHE[key] = build_program(has_rbq, has_rbk, has_b)
    return _NC_CACHE[key]


def kernel(**inputs) -> np.ndarray:
    in_maps, has_rbq, has_rbk, has_b = _host_prep(inputs)
    nc = _get_nc(has_rbq, has_rbk, has_b)
    res = bass_utils.run_bass_kernel_spmd(
        nc, in_maps, core_ids=list(range(NCORES))
    )
    outs = res.results
    full = np.empty((B, S, D), np.float32)
    for c in range(NCORES):
        b = c // 4
        s0 = (c % 4) * SQ
        full[b, s0 : s0 + SQ, :] = outs[c]["out"]
    return full


if __name__ == "__main__":
    nc = _get_nc()
    print("build + compile OK")


# revision 21
# speedup vs baseline: 1.0680x; 1.0680x over previous
"""Trainium2 Bass kernel for an MQA attention block (8 q-heads, shared K/V).

Sharding: 8 cores; core c -> batch b=c//4, query rows s0=(c%4)*512 .. +512,
all 8 heads.  K/V (full sequence, per batch) computed redundantly per core.

v5 design notes:
- The ACT engine (tanh+exp over S*S*H logits, ~2.2us per 2-key-chunk tile)
  is the hard floor; the kernel is built as one long pipeline that keeps
  ACT dense from ~20us onward.
- KV projection row-direct with concatenated [Wk|Wv] moving operand; the
  PSUM accumulator is drained to SBUF bf16 immediately (short bank hold),
  LN stats/normalize run on the SBUF copy, rope is batched 4 chunks at a
  time on DVE in 4x mode.
- Heads 0/1 attention (QK + softcap) is emitted INSIDE the KV loop as
  kT chunks become ready, so the whole KV phase hides under their ACT.
- Q projection row-direct two heads per matmul (N=256) amortizing LDW.
- Attention bias: PE identity-preload (even groups) / DVE add (odd).
- tanh writes an SBUF fp16 intermediate (not in-place PSUM) so the logits
  PSUM bank frees one ACT-pass earlier -> deeper QK pipelining.
- q-LN rstd via Newton rsqrt + q-rope on GPSIMD.
"""

import os
import sys

for _p in ("/opt/trn_rl_repo",):
    if _p not in sys.path and os.path.isdir(_p):
        sys.path.insert(0, _p)

import numpy as np
from contextlib import ExitStack

import concourse.bass as bass
import concourse.mybir as mybir
import concourse.tile as tile
from concourse import bacc
from concourse import bass_utils

F32 = mybir.dt.float32
BF16 = mybir.dt.bfloat16
F16 = mybir.dt.float16

B, S, D = 2, 2048, 1536
H, DQ, DK, DV = 8, 128, 128, 192
P = 128
SQ = S // 4          # 512 query rows per core
DC = D // P          # 12 contraction chunks
JC = S // P          # 16 key chunks
SC = SQ // P         # 4 query-row chunks
NCORES = 8
EPS_RMS = 1e-6
EPS_LN = 1e-5
SOFTCAP = 5.0
ROPE_BASE = 8192.0
HALF = DQ // 2
VW = 256             # vrow inner stride; cols 0:192 v, 192 ones
DKV = DK + DV        # 320

TT = mybir.AluOpType
AF = mybir.ActivationFunctionType


def build_program(has_rbq=False, has_rbk=False, has_b=False):
    nc = bacc.Bacc(
        "TRN2", target_bir_lowering=False, debug=False, num_devices=NCORES
    )

    def din(name, shape, dt=BF16):
        return nc.dram_tensor(name, list(shape), dt, kind="ExternalInput").ap()

    # x in device layout [P, JC, DC, P]: [p, jc, dc, col] = x.T[dc*P+p, jc*P+col]
    xT = din("xT", (P, JC, DC, P))
    xq = din("xq", (P, DC, SQ))          # per-core query-column slice of x.T
    biasT = din("biasT", (P, JC, SQ))
    c1q_t = din("c1q", (P, SC, HALF))
    s2nq_t = din("s2nq", (P, SC, HALF))
    s1q_t = din("s1q", (P, SC, HALF))
    c2q_t = din("c2q", (P, SC, HALF))
    c1k_t = din("c1k", (P, JC, HALF))
    s2nk_t = din("s2nk", (P, JC, HALF))
    s1k_t = din("s1k", (P, JC, HALF))
    c2k_t = din("c2k", (P, JC, HALF))
    wq_t = din("wq", (P, DC, H * DQ))
    wkv_t = din("wkv", (P, DC, DKV))     # [Wk | Wv] concatenated
    wo_t = din("wo", (P, DC, D))
    bor_t = din("bor", (P, D), F32)
    identb_t = din("identb", (P, P))
    if has_b:
        brow_t = din("brow", (1, DKV + H * DQ))
    if has_rbq:
        rbq_t = din("rbq", (P, SC, DQ))
    if has_rbk:
        rbk_t = din("rbk", (P, JC, DK))
    out = nc.dram_tensor("out", [SQ, D], F32, kind="ExternalOutput").ap()
    DEBUG = os.environ.get("KDEBUG", "0") == "1"
    if DEBUG:
        dbg_kT = nc.dram_tensor(
            "dbg_kT", [P, S], BF16, kind="ExternalOutput").ap()
        dbg_vrow = nc.dram_tensor(
            "dbg_vrow", [P, JC, VW], BF16, kind="ExternalOutput").ap()
        dbg_qT0 = nc.dram_tensor(
            "dbg_qT0", [P, SQ], BF16, kind="ExternalOutput").ap()
        dbg_pts0 = nc.dram_tensor(
            "dbg_pts0", [P, JC, SQ], BF16, kind="ExternalOutput").ap()
        dbg_yp0 = nc.dram_tensor(
            "dbg_yp0", [P, SC, 2 * DV], BF16, kind="ExternalOutput").ap()

    with tile.TileContext(nc) as tc, ExitStack() as ctx:
        const = ctx.enter_context(tc.tile_pool(name="const", bufs=1))
        persist = ctx.enter_context(tc.tile_pool(name="persist", bufs=1))
        qt = ctx.enter_context(tc.tile_pool(name="qt", bufs=2))
        att = ctx.enter_context(tc.tile_pool(name="att", bufs=2))
        qps_cm = tc.tile_pool(name="qps", bufs=1, space="PSUM")
        qpsp = qps_cm.__enter__()
        scr_cm = tc.tile_pool(name="scr", bufs=1, space="PSUM")
        scr = scr_cm.__enter__()
        apq_cm = tc.tile_pool(name="apq", bufs=2, space="PSUM")
        apq = apq_cm.__enter__()

        # ---------------- constants (DMA emission order matters) ----------
        identb = const.tile([P, P], BF16)
        nc.sync.dma_start(identb[:], identb_t)
        wkv_sb = const.tile([P, DC, DKV], BF16)
        nc.sync.dma_start(wkv_sb[:], wkv_t)
        xq_sb = persist.tile([P, DC, SQ], BF16)
        nc.sync.dma_start(xq_sb[:], xq)
        wq_sb = persist.tile([P, DC, 4 * DQ], BF16)
        nc.sync.dma_start(wq_sb[:], wq_t[:, :, 0 : 4 * DQ])
        biasT_sb = persist.tile([P, JC, SQ], BF16)
        nc.sync.dma_start(biasT_sb[:], biasT)

        eps_sb = const.tile([P, 1], F32)
        nc.vector.memset(eps_sb[:], EPS_LN)
        if has_b:
            brow = const.tile([1, DKV + H * DQ], BF16)
            nc.sync.dma_start(brow[:], brow_t)
            ones1 = const.tile([1, P], BF16)
            nc.vector.memset(ones1[:], 1.0)

        def load_tab(t, n, nm):
            tt = const.tile([P, n, HALF], BF16, tag=nm, name=nm)
            nc.sync.dma_start(tt[:], t)
            return tt

        c1k = load_tab(c1k_t, JC, "c1k")
        s2nk = load_tab(s2nk_t, JC, "s2nk")
        s1k = load_tab(s1k_t, JC, "s1k")
        c2k = load_tab(c2k_t, JC, "c2k")
        c1q = load_tab(c1q_t, SC, "c1q")
        s2nq = load_tab(s2nq_t, SC, "s2nq")
        s1q = load_tab(s1q_t, SC, "s1q")
        c2q = load_tab(c2q_t, SC, "c2q")
        if has_rbk:
            rbk = const.tile([P, JC, DK], BF16)
            nc.sync.dma_start(rbk[:], rbk_t)
        if has_rbq:
            rbq = const.tile([P, SC, DQ], BF16)
            nc.sync.dma_start(rbq[:], rbq_t)

        # ---------------- persistent activations ----------------
        kT_sb = persist.tile([P, S], BF16)          # rope'd k, [dk, s]
        vrow_sb = persist.tile([P, JC, VW], BF16)   # v rows + ones col
        nc.vector.memset(vrow_sb[:, :, DV : DV + 1], 1.0)
        qT = [
            persist.tile([P, SQ], BF16, tag=f"q{h}", name=f"qT{h}")
            for h in range(H)
        ]
        yp = [
            persist.tile([P, SC, 2 * DV], BF16, tag=f"yp{p}", name=f"yp{p}")
            for p in range(4)
        ]
        yT_sb = persist.tile([P, DC, SQ], BF16)

        g = (nc.gpsimd if os.environ.get("USE_GPSIMD", "1") == "1"
             else nc.vector)
        pts = {}

        # ---------------- attention primitives ----------------
        def qk_group(h, jg):
            pq = apq.tile([P, 2, SQ], F32, tag="pq", name=f"pq{h}{jg}")
            dve_bias = jg % 2 == 1
            for c in range(2):
                jc = jg * 2 + c
                if not dve_bias:
                    nc.tensor.matmul(
                        pq[:, c, :], identb[:], biasT_sb[:, jc, :],
                        start=True, stop=False,
                    )
                nc.tensor.matmul(
                    pq[:, c, :],
                    kT_sb[:, jc * P : (jc + 1) * P], qT[h][:],
                    start=dve_bias, stop=True,
                )
            if dve_bias:
                nc.vector.tensor_tensor(
                    pq[:], pq[:], biasT_sb[:, jg * 2 : jg * 2 + 2, :],
                    TT.add,
                )
            t16 = att.tile([P, 2, SQ], F16, tag="t16", name=f"t16_{h}{jg}")
            nc.scalar.activation(
                t16[:], pq[:], AF.Tanh, scale=1.0 / SOFTCAP
            )
            nc.scalar.activation(
                pts[h][:, jg * 2 : jg * 2 + 2, :], t16[:],
                AF.Exp, scale=SOFTCAP,
            )

        # -------- q pipeline (row-direct, two heads per matmul) --------
        def stage_q_proj(hp, t, parts):
            if t == 0:
                parts.append(
                    qpsp.tile([P, SC, 2, DQ], F32, tag="q_ps",
                              name=f"qps{hp}")
                )
            q_ps = parts[0]
            last = DC - 1
            h0 = 2 * hp
            w0 = (h0 % 4) * DQ
            for dc in range(DC):
                nc.tensor.matmul(
                    q_ps[:, t, :, :],
                    xq_sb[:, dc, t * P : (t + 1) * P],
                    wq_sb[:, dc, w0 : w0 + 2 * DQ],
                    start=(dc == 0), stop=(dc == last) and not has_b,
                )
            if has_b:
                nc.tensor.matmul(
                    q_ps[:, t, :, :], ones1[:],
                    brow[:, DKV + h0 * DQ : DKV + (h0 + 2) * DQ],
                    start=False, stop=True,
                )
            st6 = qt.tile([P, 2, 6], F32, tag=f"qst{t}", name=f"qst{hp}_{t}")
            for j in range(2):
                nc.vector.bn_stats(st6[:, j, :], q_ps[:, t, j, :])
            parts.append(st6)

        def stage_q_ln(hp, parts):
            q_ps = parts[0]
            qag = qt.tile([P, SC, 2, 2], F32, tag="qag", name=f"qag{hp}")
            for t in range(SC):
                for j in range(2):
                    nc.vector.bn_aggr(qag[:, t, j, :], parts[1 + t][:, j, :])
            qv = qt.tile([P, SC, 2, 1], F32, tag="qv", name=f"qv{hp}")
            qy = qt.tile([P, SC, 2, 1], F32, tag="qy", name=f"qy{hp}")
            qw2 = qt.tile([P, SC, 2, 1], F32, tag="qw2", name=f"qw2{hp}")
            g.tensor_scalar(qv[:], qag[:, :, :, 1:2], EPS_LN, None, TT.add)
            g.tensor_scalar(qy[:], qv[:], -0.5, 1.5, TT.mult, TT.add)
            for _ in range(3):
                g.tensor_tensor(qw2[:], qy[:], qy[:], TT.mult)
                g.tensor_tensor(qw2[:], qw2[:], qv[:], TT.mult)
                g.tensor_scalar(qw2[:], qw2[:], -0.5, 1.5, TT.mult, TT.add)
                g.tensor_tensor(qy[:], qy[:], qw2[:], TT.mult)
            xnq = qt.tile([P, SC, 2, DQ], BF16, tag="xnq", name=f"xnq{hp}")
            for t in range(SC):
                for j in range(2):
                    nc.vector.tensor_scalar(
                        xnq[:, t, j, :], q_ps[:, t, j, :],
                        qag[:, t, j, 0:1], qy[:, t, j, 0:1],
                        TT.subtract, TT.mult,
                    )
            for j in range(2):
                h = 2 * hp + j
                qz1 = qt.tile([P, SC, HALF], BF16, tag="qz1", name=f"qz1{h}")
                qz2 = qt.tile([P, SC, HALF], BF16, tag="qz2", name=f"qz2{h}")
                qr = qt.tile([P, SC, DQ], BF16, tag="qr", name=f"qr{h}")
                xj = xnq[:, :, j, :]
                g.tensor_tensor(qz1[:], xj[:, :, :HALF], c1q[:], TT.mult)
                g.tensor_tensor(qz2[:], xj[:, :, HALF:], s2nq[:], TT.mult)
                g.tensor_tensor(qr[:, :, :HALF], qz1[:], qz2[:], TT.add)
                g.tensor_tensor(qz1[:], xj[:, :, :HALF], s1q[:], TT.mult)
                g.tensor_tensor(qz2[:], xj[:, :, HALF:], c2q[:], TT.mult)
                g.tensor_tensor(qr[:, :, HALF:], qz1[:], qz2[:], TT.add)
                if has_rbq:
                    g.tensor_tensor(qr[:], qr[:], rbq[:], TT.add)
                for t in range(SC):
                    sct = scr.tile([P, P], BF16, tag="scb", name=f"qbt{h}{t}")
                    nc.tensor.transpose(sct[:], qr[:, t, :], identb[:])
                    nc.vector.tensor_copy(
                        qT[h][:, t * P : (t + 1) * P], sct[:]
                    )

        # =========================================================
        # KV loop with heads 0/1 attention overlapped
        # =========================================================
        for h in (0, 1):
            pts[h] = att.tile([P, JC, SQ], BF16, tag="pt", name=f"pt{h}")

        with (
            tc.tile_pool(name="kvx", bufs=2) as kvx,
            tc.tile_pool(name="kvt", bufs=2) as kvt,
            tc.tile_pool(name="kvps", bufs=1, space="PSUM") as kvps,
        ):
            parts01 = []
            parts1 = []  # pair 1 = heads 2,3

            def kv_chunk(jc):
                xt = kvx.tile([P, DC, P], BF16, tag="xt", name=f"xt{jc}")
                nc.sync.dma_start(xt[:], xT[:, jc, :, :])
                kv_ps = kvps.tile([P, DKV], F32, tag="kv_ps",
                                  name=f"kvps{jc}")
                last = DC - 1
                for dc in range(DC):
                    nc.tensor.matmul(
                        kv_ps[:], xt[:, dc, :], wkv_sb[:, dc, :],
                        start=(dc == 0), stop=(dc == last) and not has_b,
                    )
                if has_b:
                    nc.tensor.matmul(
                        kv_ps[:], ones1[:], brow[:, 0:DKV],
                        start=False, stop=True,
                    )
                # drain to SBUF immediately (frees the PSUM bank)
                kvr = kvt.tile([P, DKV], BF16, tag="kvr", name=f"kvr{jc}")
                nc.vector.tensor_copy(kvr[:], kv_ps[:])
                # stats + rstd on the bf16 copy
                kst = kvt.tile([P, 6], F32, tag="kst", name=f"kst{jc}")
                kag = kvt.tile([P, 2], F32, tag="kag", name=f"kag{jc}")
                nc.vector.bn_stats(kst[:], kvr[:, 0:DK])
                nc.vector.bn_aggr(kag[:], kst[:])
                vst = kvt.tile([P, 6], F32, tag="vst", name=f"vst{jc}")
                vag = kvt.tile([P, 2], F32, tag="vag", name=f"vag{jc}")
                nc.vector.bn_stats(vst[:], kvr[:, DK:DKV])
                nc.vector.bn_aggr(vag[:], vst[:])
                rst = kvt.tile([P, 2], F32, tag="rst", name=f"rst{jc}")
                nc.scalar.activation(
                    rst[:, 0:1], kag[:, 1:2], AF.Sqrt, bias=eps_sb[:, 0:1]
                )
                nc.scalar.activation(
                    rst[:, 1:2], vag[:, 1:2], AF.Sqrt, bias=eps_sb[:, 0:1]
                )
                nc.vector.reciprocal(rst[:], rst[:])
                # normalize k into the 4-chunk rope batch buffer, v to vrow
                if jc % 4 == 0:
                    kv_chunk.xnk4 = kvt.tile(
                        [P, 4, DK], BF16, tag="xnk4", name=f"xnk4_{jc // 4}"
                    )
                nc.vector.tensor_scalar(
                    kv_chunk.xnk4[:, jc % 4, :], kvr[:, 0:DK],
                    kag[:, 0:1], rst[:, 0:1],
                    TT.subtract, TT.mult,
                )
                nc.vector.tensor_scalar(
                    vrow_sb[:, jc, 0:DV], kvr[:, DK:DKV],
                    vag[:, 0:1], rst[:, 1:2],
                    TT.subtract, TT.mult,
                )

            def rope_batch(bi):
                # rope 4 k-chunks at once on DVE (bf16 4x mode)
                xnk4 = kv_chunk.xnk4
                jcs = slice(bi * 4, bi * 4 + 4)
                kz1 = kvt.tile([P, 4, HALF], BF16, tag="kz1", name=f"kz1{bi}")
                kz2 = kvt.tile([P, 4, HALF], BF16, tag="kz2", name=f"kz2{bi}")
                kr4 = kvt.tile([P, 4, DK], BF16, tag="kr4", name=f"kr4{bi}")
                V = nc.vector
                V.tensor_tensor(kz1[:], xnk4[:, :, :HALF], c1k[:, jcs, :],
                                TT.mult)
                V.tensor_tensor(kz2[:], xnk4[:, :, HALF:], s2nk[:, jcs, :],
                                TT.mult)
                V.tensor_tensor(kr4[:, :, :HALF], kz1[:], kz2[:], TT.add)
                V.tensor_tensor(kz1[:], xnk4[:, :, :HALF], s1k[:, jcs, :],
                                TT.mult)
                V.tensor_tensor(kz2[:], xnk4[:, :, HALF:], c2k[:, jcs, :],
                                TT.mult)
                V.tensor_tensor(kr4[:, :, HALF:], kz1[:], kz2[:], TT.add)
                if has_rbk:
                    V.tensor_tensor(kr4[:], kr4[:], rbk[:, jcs, :], TT.add)
                for t in range(4):
                    jc = bi * 4 + t
                    scb = scr.tile([P, P], BF16, tag="scb", name=f"kbt{jc}")
                    nc.tensor.transpose(scb[:], kr4[:, t, :], identb[:])
                    nc.vector.tensor_copy(
                        kT_sb[:, jc * P : (jc + 1) * P], scb[:]
                    )

            # schedule: kv chunks, rope batches, stageQ pair0/pair1,
            # heads 0/1 qk groups as kT becomes available
            for jc in range(4):
                kv_chunk(jc)
            rope_batch(0)
            for t in range(SC):
                stage_q_proj(0, t, parts01)
            stage_q_ln(0, parts01)
            for jc in range(4, 8):
                kv_chunk(jc)
                if jc == 5:
                    qk_group(0, 0)
                    qk_group(0, 1)
                if jc == 6:
                    qk_group(1, 0)
                if jc == 7:
                    qk_group(1, 1)
            rope_batch(1)
            for jc in range(8, 12):
                kv_chunk(jc)
                if jc == 8:
                    qk_group(0, 2)
                if jc == 9:
                    qk_group(0, 3)
                    stage_q_proj(1, 0, parts1)
                if jc == 10:
                    qk_group(1, 2)
                    stage_q_proj(1, 1, parts1)
                    wo_sb = persist.tile([P, DC, D], BF16)
                    nc.sync.dma_start(wo_sb[:], wo_t)
                    bor = persist.tile([P, D], F32)
                    nc.sync.dma_start(bor[:], bor_t)
                if jc == 11:
                    qk_group(1, 3)
                    stage_q_proj(1, 2, parts1)
            rope_batch(2)
            for jc in range(12, 16):
                kv_chunk(jc)
                if jc == 12:
                    qk_group(0, 4)
                if jc == 13:
                    qk_group(0, 5)
                    stage_q_proj(1, 3, parts1)
                    nc.sync.dma_start(
                        wq_sb[:], wq_t[:, :, 4 * DQ : 8 * DQ]
                    )
                if jc == 14:
                    qk_group(1, 4)
                if jc == 15:
                    qk_group(1, 5)
                    stage_q_ln(1, parts1)
            rope_batch(3)
            for jg in (6, 7):
                qk_group(0, jg)
            for jg in (6, 7):
                qk_group(1, jg)

        # =========================================================
        # heads 2..7 + PV pipeline
        # =========================================================
        with (
            tc.tile_pool(name="ay", bufs=1, space="PSUM") as ay,
        ):
            def pv_chunk(h, ic):
                y_ps = ay.tile([P, DV + 1], F32, tag="y_ps",
                               name=f"yps{h}{ic}")
                for jc in range(JC):
                    nc.tensor.matmul(
                        y_ps[:],
                        pts[h][:, jc, ic * P : (ic + 1) * P],
                        vrow_sb[:, jc, : DV + 1],
                        start=(jc == 0), stop=(jc == JC - 1),
                    )
                rcp = att.tile([P, 1], F32, tag="rcp", name=f"rcp{h}{ic}")
                nc.vector.reciprocal(rcp[:], y_ps[:, DV : DV + 1])
                nc.vector.tensor_scalar_mul(
                    yp[h // 2][:, ic, (h % 2) * DV : (h % 2 + 1) * DV],
                    y_ps[:, :DV], rcp[:, 0:1],
                )

            def oproj_transposes(p):
                for sc in range(SC):
                    for fcl in range(3):
                        fc = 3 * p + fcl
                        scb = scr.tile([P, P], BF16, tag="scb",
                                       name=f"so{p}{sc}{fcl}")
                        nc.tensor.transpose(
                            scb[:],
                            yp[p][:, sc, fcl * P : (fcl + 1) * P],
                            identb[:],
                        )
                        nc.vector.tensor_copy(
                            yT_sb[:, fc, sc * P : (sc + 1) * P], scb[:]
                        )

            # pv(0) immediately after the overlap section so the pts[0]
            # buffer (reused by pts[2]) frees before head 2's exps
            for ic in range(SC):
                pv_chunk(0, ic)

            parts_by_pair = {}
            for h in range(2, H):
                pts[h] = att.tile([P, JC, SQ], BF16, tag="pt", name=f"pt{h}")
                fillers = []
                hp = h // 2 + 1
                if hp < 4:
                    if h % 2 == 0:
                        partsn = []
                        parts_by_pair[hp] = partsn
                        for t in range(SC):
                            fillers.append(
                                lambda hp=hp, t=t, pn=partsn: stage_q_proj(
                                    hp, t, pn
                                )
                            )
                    else:
                        fillers.append(
                            lambda hp=hp: stage_q_ln(hp, parts_by_pair[hp])
                        )
                # pv of the previous head must fully emit during this head
                # (pts pool has 2 buffers)
                for ic in range(SC):
                    fillers.append(lambda h=h, ic=ic: pv_chunk(h - 1, ic))
                if h >= 4 and h % 2 == 0:
                    fillers.append(lambda p=(h - 4) // 2: oproj_transposes(p))

                nf = len(fillers)
                done = 0
                for jg in range(JC // 2):
                    qk_group(h, jg)
                    want = (nf * (jg + 1) + 7) // 8
                    while done < want:
                        fillers[done]()
                        done += 1
            # tail
            for ic in range(SC):
                pv_chunk(H - 1, ic)
            oproj_transposes(2)
            oproj_transposes(3)
            if DEBUG:
                nc.sync.dma_start(dbg_kT[:], kT_sb[:])
                nc.sync.dma_start(dbg_vrow[:], vrow_sb[:])
                nc.sync.dma_start(dbg_qT0[:], qT[0][:])
                nc.sync.dma_start(dbg_pts0[:], pts[0][:])
                nc.sync.dma_start(dbg_yp0[:], yp[0][:])

        apq_cm.__exit__(None, None, None)
        scr_cm.__exit__(None, None, None)
        qps_cm.__exit__(None, None, None)

        # =========================================================
        # Output projection (bf16)
        # =========================================================
        with (
            tc.tile_pool(name="od", bufs=2) as od,
            tc.tile_pool(name="ops", bufs=2, space="PSUM") as ops,
        ):
            for sc in range(SC):
                o_ps = ops.tile([P, D], F32, tag="o_ps", name=f"ops{sc}")
                for fc in range(DC):
                    for n in range(D // 512):
                        nc.tensor.matmul(
                            o_ps[:, n * 512 : (n + 1) * 512],
                            yT_sb[:, fc, sc * P : (sc + 1) * P],
                            wo_sb[:, fc, n * 512 : (n + 1) * 512],
                            start=(fc == 0), stop=(fc == DC - 1),
                        )
                o_sb = od.tile([P, D], F32, tag="o_sb", name=f"osb{sc}")
                nc.vector.tensor_tensor(o_sb[:], o_ps[:], bor[:], TT.add)
                nc.sync.dma_start(out[sc * P : (sc + 1) * P, :], o_sb[:])

    nc.compile()
    return nc


def _host_prep(inputs):
    import ml_dtypes

    f32 = np.float32
    bf16 = ml_dtypes.bfloat16
    x = np.asarray(inputs["x"], f32)
    bias = np.asarray(inputs["attention_bias"], f32)
    g1 = np.asarray(inputs["g1"], f32)
    b1 = np.asarray(inputs["b1"], f32)
    rr1 = np.asarray(inputs["rrms1"], f32)
    Wq = np.asarray(inputs["Wq"], f32)
    Wk = np.asarray(inputs["Wk"], f32)
    Wv = np.asarray(inputs["Wv"], f32)
    qg = np.asarray(inputs["qg"], f32)
    qb = np.asarray(inputs["qb"], f32)
    kg = np.asarray(inputs["kg"], f32)
    kb = np.asarray(inputs["kb"], f32)
    vg = np.asarray(inputs["vg"], f32)
    vb = np.asarray(inputs["vb"], f32)
    Wo = np.asarray(inputs["Wo"], f32)
    bo = np.asarray(inputs["bo"], f32)
    g2 = np.asarray(inputs["g2"], f32)
    b2 = np.asarray(inputs["b2"], f32)
    rr2 = np.asarray(inputs["rrms2"], f32)

    scale1 = (g1 * (1.0 / np.sqrt(rr1 + EPS_RMS))).astype(f32)
    Wq_e = (Wq * scale1[:, None]).astype(f32)
    Wk_e = (Wk * scale1[:, None]).astype(f32)
    Wv_e = (Wv * scale1[:, None]).astype(f32)
    bq_row = (b1 @ Wq).astype(f32)      # [H*DQ]
    bk_row = (b1 @ Wk).astype(f32)      # [DK]
    bv_row = (b1 @ Wv).astype(f32)      # [DV]
    sc_q = f32(DQ) ** f32(-0.5)
    qg_e = (qg * sc_q).astype(f32)
    qb_e = (qb * sc_q).astype(f32)

    scale2 = (g2 * (1.0 / np.sqrt(rr2 + EPS_RMS))).astype(f32)
    vg_rep = np.tile(vg, H)                      # [H*DV]
    Wo_e = (Wo * vg_rep[:, None] * scale2[None, :]).astype(f32)
    vb_fold = (np.tile(vb, H) @ Wo).astype(f32)  # [D]
    bo_e = ((bo + vb_fold) * scale2 + b2).astype(f32)

    freqs = (
        1.0 / (ROPE_BASE ** (np.arange(HALF, dtype=f32) / HALF))
    ).astype(f32)
    ang = np.arange(S, dtype=f32)[:, None] * freqs[None, :]
    cos = np.cos(ang).astype(f32)                        # [S, 64]
    sin = np.sin(ang).astype(f32)

    c1k = (cos * kg[None, :HALF]).astype(bf16)
    s2nk = (-sin * kg[None, HALF:]).astype(bf16)
    s1k = (sin * kg[None, :HALF]).astype(bf16)
    c2k = (cos * kg[None, HALF:]).astype(bf16)

    rbk_f = np.concatenate(
        [cos * kb[None, :HALF] - sin * kb[None, HALF:],
         sin * kb[None, :HALF] + cos * kb[None, HALF:]], axis=1
    ).astype(f32)
    rbq_f = np.concatenate(
        [cos * qb_e[None, :HALF] - sin * qb_e[None, HALF:],
         sin * qb_e[None, :HALF] + cos * qb_e[None, HALF:]], axis=1
    ).astype(f32)
    has_rbk = bool(np.any(rbk_f))
    has_rbq = bool(np.any(rbq_f))
    has_b = bool(np.any(bq_row) or np.any(bk_row) or np.any(bv_row))

    def dev3(a, n):
        return np.ascontiguousarray(
            a.reshape(n, P, a.shape[-1]).transpose(1, 0, 2)
        )

    rep = lambda v: np.ascontiguousarray(
        np.broadcast_to(v[None, :], (P, v.shape[0]))
    )
    wkv = np.concatenate([Wk_e, Wv_e], axis=1)          # [D, DK+DV]
    shared = {
        "c1k": dev3(c1k, JC),
        "s2nk": dev3(s2nk, JC),
        "s1k": dev3(s1k, JC),
        "c2k": dev3(c2k, JC),
        "wq": dev3(Wq_e.astype(bf16), DC),
        "wkv": dev3(wkv.astype(bf16), DC),
        "wo": dev3(Wo_e.astype(bf16), DC),
        "bor": rep(bo_e),
        "identb": np.eye(P, dtype=bf16),
    }
    if has_b:
        shared["brow"] = np.concatenate(
            [bk_row, bv_row, bq_row]
        ).astype(bf16)[None, :]
    if has_rbk:
        shared["rbk"] = dev3(rbk_f.astype(bf16), JC)

    xdev = []
    for b in range(B):
        xTb = np.ascontiguousarray(x[b].T).astype(bf16)      # [D, S]
        xdev.append(np.ascontiguousarray(
            xTb.reshape(DC, P, JC, P).transpose(1, 2, 0, 3)
        ))
    xTs = [np.ascontiguousarray(x[b].T) for b in range(B)]
    in_maps = []
    for c in range(NCORES):
        b = c // 4
        s0 = (c % 4) * SQ
        m = dict(shared)
        m["xT"] = xdev[b]
        m["xq"] = dev3(xTs[b][:, s0 : s0 + SQ].astype(bf16), DC)
        m["biasT"] = dev3(bias[0, 0, s0 : s0 + SQ, :].T.astype(bf16), JC)
        m["c1q"] = dev3(
            (cos[s0 : s0 + SQ] * qg_e[None, :HALF]).astype(bf16), SC
        )
        m["s2nq"] = dev3(
            (-sin[s0 : s0 + SQ] * qg_e[None, HALF:]).astype(bf16), SC
        )
        m["s1q"] = dev3(
            (sin[s0 : s0 + SQ] * qg_e[None, :HALF]).astype(bf16), SC
        )
        m["c2q"] = dev3(
            (cos[s0 : s0 + SQ] * qg_e[None, HALF:]).astype(bf16), SC
        )
        if has_rbq:
            m["rbq"] = dev3(rbq_f[s0 : s0 + SQ].astype(bf16), SC)
        in_maps.append(m)
    return in_maps, has_rbq, has_rbk, has_b


_NC_CACHE = {}


def _get_nc(has_rbq=False, has_rbk=False, has_b=False):
    key = (has_rbq, has_rbk, has_b)
    if key not in _NC_CACHE:
        _NC_CACHE[key] = build_program(has_rbq, has_rbk, has_b)
    return _NC_CACHE[key]


def kernel(**inputs) -> np.ndarray:
    in_maps, has_rbq, has_rbk, has_b = _host_prep(inputs)
    nc = _get_nc(has_rbq, has_rbk, has_b)
    res = bass_utils.run_bass_kernel_spmd(
        nc, in_maps, core_ids=list(range(NCORES))
    )
    outs = res.results
    full = np.empty((B, S, D), np.float32)
    for c in range(NCORES):
        b = c // 4
        s0 = (c % 4) * SQ
        full[b, s0 : s0 + SQ, :] = outs[c]["out"]
    return full


if __name__ == "__main__":
    nc = _get_nc()
    print("build + compile OK")


# revision 23
# speedup vs baseline: 1.1616x; 1.0876x over previous
"""Trainium2 Bass kernel for an MQA attention block (8 q-heads, shared K/V).

Sharding: 8 cores; core c -> batch b=c//4, query rows s0=(c%4)*512 .. +512,
all 8 heads.  K/V (full sequence, per batch) computed redundantly per core.

v5 design notes:
- The ACT engine (tanh+exp over S*S*H logits, ~2.2us per 2-key-chunk tile)
  is the hard floor; the kernel is built as one long pipeline that keeps
  ACT dense from ~20us onward.
- KV projection row-direct with concatenated [Wk|Wv] moving operand; the
  PSUM accumulator is drained to SBUF bf16 immediately (short bank hold),
  LN stats/normalize run on the SBUF copy, rope is batched 4 chunks at a
  time on DVE in 4x mode.
- Heads 0/1 attention (QK + softcap) is emitted INSIDE the KV loop as
  kT chunks become ready, so the whole KV phase hides under their ACT.
- Q projection row-direct two heads per matmul (N=256) amortizing LDW.
- Attention bias: PE identity-preload (even groups) / DVE add (odd).
- tanh writes an SBUF fp16 intermediate (not in-place PSUM) so the logits
  PSUM bank frees one ACT-pass earlier -> deeper QK pipelining.
- q-LN rstd via Newton rsqrt + q-rope on GPSIMD.
"""

import os
import sys

for _p in ("/opt/trn_rl_repo",):
    if _p not in sys.path and os.path.isdir(_p):
        sys.path.insert(0, _p)

import numpy as np
from contextlib import ExitStack

import concourse.bass as bass
import concourse.mybir as mybir
import concourse.tile as tile
from concourse import bacc
from concourse import bass_utils

F32 = mybir.dt.float32
BF16 = mybir.dt.bfloat16
F16 = mybir.dt.float16

B, S, D = 2, 2048, 1536
H, DQ, DK, DV = 8, 128, 128, 192
P = 128
SQ = S // 4          # 512 query rows per core
DC = D // P          # 12 contraction chunks
JC = S // P          # 16 key chunks
SC = SQ // P         # 4 query-row chunks
NCORES = 8
EPS_RMS = 1e-6
EPS_LN = 1e-5
SOFTCAP = 5.0
ROPE_BASE = 8192.0
HALF = DQ // 2
VW = 256             # vrow inner stride; cols 0:192 v, 192 ones
DKV = DK + DV        # 320

TT = mybir.AluOpType
AF = mybir.ActivationFunctionType


def build_program(has_rbq=False, has_rbk=False, has_b=False):
    nc = bacc.Bacc(
        "TRN2", target_bir_lowering=False, debug=False, num_devices=NCORES
    )

    def din(name, shape, dt=BF16):
        return nc.dram_tensor(name, list(shape), dt, kind="ExternalInput").ap()

    # x in device layout [P, JC, DC, P]: [p, jc, dc, col] = x.T[dc*P+p, jc*P+col]
    xT = din("xT", (P, JC, DC, P))
    xq = din("xq", (P, DC, SQ))          # per-core query-column slice of x.T
    biasT = din("biasT", (P, JC, SQ))
    c1q_t = din("c1q", (P, SC, HALF))
    s2nq_t = din("s2nq", (P, SC, HALF))
    s1q_t = din("s1q", (P, SC, HALF))
    c2q_t = din("c2q", (P, SC, HALF))
    c1k_t = din("c1k", (P, JC, HALF))
    s2nk_t = din("s2nk", (P, JC, HALF))
    s1k_t = din("s1k", (P, JC, HALF))
    c2k_t = din("c2k", (P, JC, HALF))
    wq_t = din("wq", (P, DC, H * DQ))
    wkv_t = din("wkv", (P, DC, DKV))     # [Wk | Wv] concatenated
    wo_t = din("wo", (P, DC, D))
    bor_t = din("bor", (P, D), F32)
    identb_t = din("identb", (P, P))
    if has_b:
        brow_t = din("brow", (1, DKV + H * DQ))
    if has_rbq:
        rbq_t = din("rbq", (P, SC, DQ))
    if has_rbk:
        rbk_t = din("rbk", (P, JC, DK))
    out = nc.dram_tensor("out", [SQ, D], F32, kind="ExternalOutput").ap()
    DEBUG = os.environ.get("KDEBUG", "0") == "1"
    if DEBUG:
        dbg_kT = nc.dram_tensor(
            "dbg_kT", [P, S], BF16, kind="ExternalOutput").ap()
        dbg_vrow = nc.dram_tensor(
            "dbg_vrow", [P, JC, VW], BF16, kind="ExternalOutput").ap()
        dbg_qT0 = nc.dram_tensor(
            "dbg_qT0", [P, SQ], BF16, kind="ExternalOutput").ap()
        dbg_pts0 = nc.dram_tensor(
            "dbg_pts0", [P, JC, SQ], BF16, kind="ExternalOutput").ap()
        dbg_yp0 = nc.dram_tensor(
            "dbg_yp0", [P, SC, 2 * DV], BF16, kind="ExternalOutput").ap()

    with tile.TileContext(nc) as tc, ExitStack() as ctx:
        const = ctx.enter_context(tc.tile_pool(name="const", bufs=1))
        persist = ctx.enter_context(tc.tile_pool(name="persist", bufs=1))
        qt = ctx.enter_context(tc.tile_pool(name="qt", bufs=2))
        att = ctx.enter_context(tc.tile_pool(name="att", bufs=2))
        qps_cm = tc.tile_pool(name="qps", bufs=1, space="PSUM")
        qpsp = qps_cm.__enter__()
        scr_cm = tc.tile_pool(name="scr", bufs=1, space="PSUM")
        scr = scr_cm.__enter__()
        apq_cm = tc.tile_pool(name="apq", bufs=2, space="PSUM")
        apq = apq_cm.__enter__()

        # ---------------- constants (DMA emission order matters) ----------
        identb = const.tile([P, P], BF16)
        nc.sync.dma_start(identb[:], identb_t)
        wkv_sb = const.tile([P, DC, DKV], BF16)
        nc.sync.dma_start(wkv_sb[:], wkv_t)
        xq_sb = persist.tile([P, DC, SQ], BF16)
        nc.sync.dma_start(xq_sb[:], xq)
        wq_sb = persist.tile([P, DC, 4 * DQ], BF16)
        nc.sync.dma_start(wq_sb[:], wq_t[:, :, 0 : 4 * DQ])
        biasT_sb = persist.tile([P, JC, SQ], BF16)
        nc.sync.dma_start(biasT_sb[:], biasT)

        eps_sb = const.tile([P, 1], F32)
        nc.vector.memset(eps_sb[:], EPS_LN)
        if has_b:
            brow = const.tile([1, DKV + H * DQ], BF16)
            nc.sync.dma_start(brow[:], brow_t)
            ones1 = const.tile([1, P], BF16)
            nc.vector.memset(ones1[:], 1.0)

        def load_tab(t, n, nm):
            tt = const.tile([P, n, HALF], BF16, tag=nm, name=nm)
            nc.sync.dma_start(tt[:], t)
            return tt

        c1k = load_tab(c1k_t, JC, "c1k")
        s2nk = load_tab(s2nk_t, JC, "s2nk")
        s1k = load_tab(s1k_t, JC, "s1k")
        c2k = load_tab(c2k_t, JC, "c2k")
        c1q = load_tab(c1q_t, SC, "c1q")
        s2nq = load_tab(s2nq_t, SC, "s2nq")
        s1q = load_tab(s1q_t, SC, "s1q")
        c2q = load_tab(c2q_t, SC, "c2q")
        if has_rbk:
            rbk = const.tile([P, JC, DK], BF16)
            nc.sync.dma_start(rbk[:], rbk_t)
        if has_rbq:
            rbq = const.tile([P, SC, DQ], BF16)
            nc.sync.dma_start(rbq[:], rbq_t)

        # ---------------- persistent activations ----------------
        kT_sb = persist.tile([P, S], BF16)          # rope'd k, [dk, s]
        vrow_sb = persist.tile([P, JC, VW], BF16)   # v rows + ones col
        nc.vector.memset(vrow_sb[:, :, DV : DV + 1], 1.0)
        qT = [
            persist.tile([P, SQ], BF16, tag=f"q{h}", name=f"qT{h}")
            for h in range(H)
        ]
        yp = [
            persist.tile([P, SC, 2 * DV], BF16, tag=f"yp{p}", name=f"yp{p}")
            for p in range(4)
        ]
        yT_sb = persist.tile([P, DC, SQ], BF16)

        g = (nc.gpsimd if os.environ.get("USE_GPSIMD", "1") == "1"
             else nc.vector)
        pts = {}

        # ---------------- attention primitives ----------------
        def qk_group(h, jg):
            pq = apq.tile([P, 2, SQ], F32, tag="pq", name=f"pq{h}{jg}")
            dve_bias = jg % 2 == 1
            for c in range(2):
                jc = jg * 2 + c
                if not dve_bias:
                    nc.tensor.matmul(
                        pq[:, c, :], identb[:], biasT_sb[:, jc, :],
                        start=True, stop=False,
                    )
                nc.tensor.matmul(
                    pq[:, c, :],
                    kT_sb[:, jc * P : (jc + 1) * P], qT[h][:],
                    start=dve_bias, stop=True,
                )
            if dve_bias:
                nc.vector.tensor_tensor(
                    pq[:], pq[:], biasT_sb[:, jg * 2 : jg * 2 + 2, :],
                    TT.add,
                )
            t16 = att.tile([P, 2, SQ], F16, tag="t16", name=f"t16_{h}{jg}")
            nc.scalar.activation(
                t16[:], pq[:], AF.Tanh, scale=1.0 / SOFTCAP
            )
            nc.scalar.activation(
                pts[h][:, jg * 2 : jg * 2 + 2, :], t16[:],
                AF.Exp, scale=SOFTCAP,
            )

        # -------- q pipeline (row-direct, two heads per matmul) --------
        def stage_q_proj(hp, t, parts):
            if t == 0:
                parts.append(
                    qpsp.tile([P, SC, 2, DQ], F32, tag="q_ps",
                              name=f"qps{hp}")
                )
            q_ps = parts[0]
            last = DC - 1
            h0 = 2 * hp
            w0 = (h0 % 4) * DQ
            for dc in range(DC):
                nc.tensor.matmul(
                    q_ps[:, t, :, :],
                    xq_sb[:, dc, t * P : (t + 1) * P],
                    wq_sb[:, dc, w0 : w0 + 2 * DQ],
                    start=(dc == 0), stop=(dc == last) and not has_b,
                )
            if has_b:
                nc.tensor.matmul(
                    q_ps[:, t, :, :], ones1[:],
                    brow[:, DKV + h0 * DQ : DKV + (h0 + 2) * DQ],
                    start=False, stop=True,
                )
            st6 = qt.tile([P, 2, 6], F32, tag=f"qst{t}", name=f"qst{hp}_{t}")
            for j in range(2):
                nc.vector.bn_stats(st6[:, j, :], q_ps[:, t, j, :])
            parts.append(st6)

        def stage_q_ln(hp, parts):
            q_ps = parts[0]
            qag = qt.tile([P, SC, 2, 2], F32, tag="qag", name=f"qag{hp}")
            for t in range(SC):
                for j in range(2):
                    nc.vector.bn_aggr(qag[:, t, j, :], parts[1 + t][:, j, :])
            qv = qt.tile([P, SC, 2, 1], F32, tag="qv", name=f"qv{hp}")
            qy = qt.tile([P, SC, 2, 1], F32, tag="qy", name=f"qy{hp}")
            qw2 = qt.tile([P, SC, 2, 1], F32, tag="qw2", name=f"qw2{hp}")
            g.tensor_scalar(qv[:], qag[:, :, :, 1:2], EPS_LN, None, TT.add)
            g.tensor_scalar(qy[:], qv[:], -0.5, 1.5, TT.mult, TT.add)
            for _ in range(3):
                g.tensor_tensor(qw2[:], qy[:], qy[:], TT.mult)
                g.tensor_tensor(qw2[:], qw2[:], qv[:], TT.mult)
                g.tensor_scalar(qw2[:], qw2[:], -0.5, 1.5, TT.mult, TT.add)
                g.tensor_tensor(qy[:], qy[:], qw2[:], TT.mult)
            xnq = qt.tile([P, SC, 2, DQ], BF16, tag="xnq", name=f"xnq{hp}")
            for t in range(SC):
                for j in range(2):
                    nc.vector.tensor_scalar(
                        xnq[:, t, j, :], q_ps[:, t, j, :],
                        qag[:, t, j, 0:1], qy[:, t, j, 0:1],
                        TT.subtract, TT.mult,
                    )
            for j in range(2):
                h = 2 * hp + j
                qz1 = qt.tile([P, SC, HALF], BF16, tag="qz1", name=f"qz1{h}")
                qz2 = qt.tile([P, SC, HALF], BF16, tag="qz2", name=f"qz2{h}")
                qr = qt.tile([P, SC, DQ], BF16, tag="qr", name=f"qr{h}")
                xj = xnq[:, :, j, :]
                g.tensor_tensor(qz1[:], xj[:, :, :HALF], c1q[:], TT.mult)
                g.tensor_tensor(qz2[:], xj[:, :, HALF:], s2nq[:], TT.mult)
                g.tensor_tensor(qr[:, :, :HALF], qz1[:], qz2[:], TT.add)
                g.tensor_tensor(qz1[:], xj[:, :, :HALF], s1q[:], TT.mult)
                g.tensor_tensor(qz2[:], xj[:, :, HALF:], c2q[:], TT.mult)
                g.tensor_tensor(qr[:, :, HALF:], qz1[:], qz2[:], TT.add)
                if has_rbq:
                    g.tensor_tensor(qr[:], qr[:], rbq[:], TT.add)
                for t in range(SC):
                    sct = scr.tile([P, P], BF16, tag="scb", name=f"qbt{h}{t}")
                    nc.tensor.transpose(sct[:], qr[:, t, :], identb[:])
                    nc.vector.tensor_copy(
                        qT[h][:, t * P : (t + 1) * P], sct[:]
                    )

        # =========================================================
        # KV loop with heads 0/1 attention overlapped
        # =========================================================
        for h in (0, 1):
            pts[h] = att.tile([P, JC, SQ], BF16, tag="pt", name=f"pt{h}")

        with (
            tc.tile_pool(name="kvx", bufs=2) as kvx,
            tc.tile_pool(name="kvt", bufs=2) as kvt,
            tc.tile_pool(name="kvrp", bufs=5) as kvrp,
            tc.tile_pool(name="kvps", bufs=1, space="PSUM") as kvps,
        ):
            parts01 = []
            parts1 = []  # pair 1 = heads 2,3

            def kv_chunk(jc):
                xt = kvx.tile([P, DC, P], BF16, tag="xt", name=f"xt{jc}")
                nc.sync.dma_start(xt[:], xT[:, jc, :, :])
                kv_ps = kvps.tile([P, DKV], F32, tag="kv_ps",
                                  name=f"kvps{jc}")
                last = DC - 1
                for dc in range(DC):
                    nc.tensor.matmul(
                        kv_ps[:], xt[:, dc, :], wkv_sb[:, dc, :],
                        start=(dc == 0), stop=(dc == last) and not has_b,
                    )
                if has_b:
                    nc.tensor.matmul(
                        kv_ps[:], ones1[:], brow[:, 0:DKV],
                        start=False, stop=True,
                    )
                # drain to SBUF immediately (frees the PSUM bank)
                kvr = kvrp.tile([P, DKV], BF16, tag="kvr", name=f"kvr{jc}")
                nc.vector.tensor_copy(kvr[:], kv_ps[:])
                kv_chunk.kvrs[jc % 4] = kvr
                # stats on the bf16 copy; (mean, var) pairs into batch tile
                if jc % 4 == 0:
                    kv_chunk.agg4 = kvt.tile(
                        [P, 4, 2, 2], F32, tag="agg4", name=f"agg4_{jc // 4}"
                    )
                kst = kvt.tile([P, 6], F32, tag="kst", name=f"kst{jc}")
                nc.vector.bn_stats(kst[:], kvr[:, 0:DK])
                nc.vector.bn_aggr(kv_chunk.agg4[:, jc % 4, 0, :], kst[:])
                vst = kvt.tile([P, 6], F32, tag="vst", name=f"vst{jc}")
                nc.vector.bn_stats(vst[:], kvr[:, DK:DKV])
                nc.vector.bn_aggr(kv_chunk.agg4[:, jc % 4, 1, :], vst[:])

            kv_chunk.kvrs = [None] * 4

            def rope_batch(bi):
                # newton rsqrt for the 4 chunks' k/v (no ACT table switch),
                # then normalize and rope on DVE (bf16 4x mode)
                agg4 = kv_chunk.agg4
                nv = kvt.tile([P, 4, 2, 1], F32, tag="nv", name=f"nv{bi}")
                ny = kvt.tile([P, 4, 2, 1], F32, tag="ny", name=f"ny{bi}")
                nw = kvt.tile([P, 4, 2, 1], F32, tag="nw", name=f"nw{bi}")
                g.tensor_scalar(nv[:], agg4[:, :, :, 1:2], EPS_LN, None,
                                TT.add)
                g.tensor_scalar(ny[:], nv[:], -0.5, 1.5, TT.mult, TT.add)
                for _ in range(3):
                    g.tensor_tensor(nw[:], ny[:], ny[:], TT.mult)
                    g.tensor_tensor(nw[:], nw[:], nv[:], TT.mult)
                    g.tensor_scalar(nw[:], nw[:], -0.5, 1.5, TT.mult, TT.add)
                    g.tensor_tensor(ny[:], ny[:], nw[:], TT.mult)
                xnk4 = kvt.tile([P, 4, DK], BF16, tag="xnk4", name=f"xnk4_{bi}")
                for t in range(4):
                    jc = bi * 4 + t
                    kvr = kv_chunk.kvrs[t]
                    nc.vector.tensor_scalar(
                        xnk4[:, t, :], kvr[:, 0:DK],
                        agg4[:, t, 0, 0:1], ny[:, t, 0, 0:1],
                        TT.subtract, TT.mult,
                    )
                    nc.vector.tensor_scalar(
                        vrow_sb[:, jc, 0:DV], kvr[:, DK:DKV],
                        agg4[:, t, 1, 0:1], ny[:, t, 1, 0:1],
                        TT.subtract, TT.mult,
                    )
                jcs = slice(bi * 4, bi * 4 + 4)
                kz1 = kvt.tile([P, 4, HALF], BF16, tag="kz1", name=f"kz1{bi}")
                kz2 = kvt.tile([P, 4, HALF], BF16, tag="kz2", name=f"kz2{bi}")
                kr4 = kvt.tile([P, 4, DK], BF16, tag="kr4", name=f"kr4{bi}")
                V = nc.vector
                V.tensor_tensor(kz1[:], xnk4[:, :, :HALF], c1k[:, jcs, :],
                                TT.mult)
                V.tensor_tensor(kz2[:], xnk4[:, :, HALF:], s2nk[:, jcs, :],
                                TT.mult)
                V.tensor_tensor(kr4[:, :, :HALF], kz1[:], kz2[:], TT.add)
                V.tensor_tensor(kz1[:], xnk4[:, :, :HALF], s1k[:, jcs, :],
                                TT.mult)
                V.tensor_tensor(kz2[:], xnk4[:, :, HALF:], c2k[:, jcs, :],
                                TT.mult)
                V.tensor_tensor(kr4[:, :, HALF:], kz1[:], kz2[:], TT.add)
                if has_rbk:
                    V.tensor_tensor(kr4[:], kr4[:], rbk[:, jcs, :], TT.add)
                for t in range(4):
                    jc = bi * 4 + t
                    scb = scr.tile([P, P], BF16, tag="scb", name=f"kbt{jc}")
                    nc.tensor.transpose(scb[:], kr4[:, t, :], identb[:])
                    nc.vector.tensor_copy(
                        kT_sb[:, jc * P : (jc + 1) * P], scb[:]
                    )

            # schedule: kv chunks, rope batches, stageQ pair0/pair1,
            # heads 0/1 qk groups as kT becomes available
            for jc in range(4):
                kv_chunk(jc)
            rope_batch(0)
            for t in range(SC):
                stage_q_proj(0, t, parts01)
            stage_q_ln(0, parts01)
            for jc in range(4, 8):
                kv_chunk(jc)
                if jc == 5:
                    qk_group(0, 0)
                    qk_group(0, 1)
                if jc == 6:
                    qk_group(1, 0)
                if jc == 7:
                    qk_group(1, 1)
            rope_batch(1)
            for jc in range(8, 12):
                kv_chunk(jc)
                if jc == 8:
                    qk_group(0, 2)
                if jc == 9:
                    qk_group(0, 3)
                    stage_q_proj(1, 0, parts1)
                if jc == 10:
                    qk_group(1, 2)
                    stage_q_proj(1, 1, parts1)
                    wo_sb = persist.tile([P, DC, D], BF16)
                    nc.sync.dma_start(wo_sb[:], wo_t)
                    bor = persist.tile([P, D], F32)
                    nc.sync.dma_start(bor[:], bor_t)
                if jc == 11:
                    qk_group(1, 3)
                    stage_q_proj(1, 2, parts1)
            rope_batch(2)
            for jc in range(12, 16):
                kv_chunk(jc)
                if jc == 12:
                    qk_group(0, 4)
                if jc == 13:
                    qk_group(0, 5)
                    stage_q_proj(1, 3, parts1)
                    nc.sync.dma_start(
                        wq_sb[:], wq_t[:, :, 4 * DQ : 8 * DQ]
                    )
                if jc == 14:
                    qk_group(1, 4)
                if jc == 15:
                    qk_group(1, 5)
                    stage_q_ln(1, parts1)
            rope_batch(3)
            for jg in (6, 7):
                qk_group(0, jg)
            for jg in (6, 7):
                qk_group(1, jg)

        # =========================================================
        # heads 2..7 + PV pipeline
        # =========================================================
        with (
            tc.tile_pool(name="ay", bufs=1, space="PSUM") as ay,
        ):
            def pv_chunk(h, ic):
                y_ps = ay.tile([P, DV + 1], F32, tag="y_ps",
                               name=f"yps{h}{ic}")
                for jc in range(JC):
                    nc.tensor.matmul(
                        y_ps[:],
                        pts[h][:, jc, ic * P : (ic + 1) * P],
                        vrow_sb[:, jc, : DV + 1],
                        start=(jc == 0), stop=(jc == JC - 1),
                    )
                rcp = att.tile([P, 1], F32, tag="rcp", name=f"rcp{h}{ic}")
                nc.vector.reciprocal(rcp[:], y_ps[:, DV : DV + 1])
                nc.vector.tensor_scalar_mul(
                    yp[h // 2][:, ic, (h % 2) * DV : (h % 2 + 1) * DV],
                    y_ps[:, :DV], rcp[:, 0:1],
                )

            def oproj_transposes(p):
                for sc in range(SC):
                    for fcl in range(3):
                        fc = 3 * p + fcl
                        scb = scr.tile([P, P], BF16, tag="scb",
                                       name=f"so{p}{sc}{fcl}")
                        nc.tensor.transpose(
                            scb[:],
                            yp[p][:, sc, fcl * P : (fcl + 1) * P],
                            identb[:],
                        )
                        nc.vector.tensor_copy(
                            yT_sb[:, fc, sc * P : (sc + 1) * P], scb[:]
                        )

            # pv(0) immediately after the overlap section so the pts[0]
            # buffer (reused by pts[2]) frees before head 2's exps
            for ic in range(SC):
                pv_chunk(0, ic)

            parts_by_pair = {}
            for h in range(2, H):
                pts[h] = att.tile([P, JC, SQ], BF16, tag="pt", name=f"pt{h}")
                fillers = []
                hp = h // 2 + 1
                if hp < 4:
                    if h % 2 == 0:
                        partsn = []
                        parts_by_pair[hp] = partsn
                        for t in range(SC):
                            fillers.append(
                                lambda hp=hp, t=t, pn=partsn: stage_q_proj(
                                    hp, t, pn
                                )
                            )
                    else:
                        fillers.append(
                            lambda hp=hp: stage_q_ln(hp, parts_by_pair[hp])
                        )
                # pv of the previous head must fully emit during this head
                # (pts pool has 2 buffers)
                for ic in range(SC):
                    fillers.append(lambda h=h, ic=ic: pv_chunk(h - 1, ic))
                if 4 <= h <= 6:
                    fillers.append(lambda p=h - 4: oproj_transposes(p))

                nf = len(fillers)
                done = 0
                for jg in range(JC // 2):
                    qk_group(h, jg)
                    want = (nf * (jg + 1) + 7) // 8
                    while done < want:
                        fillers[done]()
                        done += 1
            # tail
            for ic in range(SC):
                pv_chunk(H - 1, ic)
            oproj_transposes(3)
            if DEBUG:
                nc.sync.dma_start(dbg_kT[:], kT_sb[:])
                nc.sync.dma_start(dbg_vrow[:], vrow_sb[:])
                nc.sync.dma_start(dbg_qT0[:], qT[0][:])
                nc.sync.dma_start(dbg_pts0[:], pts[0][:])
                nc.sync.dma_start(dbg_yp0[:], yp[0][:])

        apq_cm.__exit__(None, None, None)
        scr_cm.__exit__(None, None, None)
        qps_cm.__exit__(None, None, None)

        # =========================================================
        # Output projection (bf16)
        # =========================================================
        with (
            tc.tile_pool(name="od", bufs=2) as od,
            tc.tile_pool(name="ops", bufs=2, space="PSUM") as ops,
        ):
            for sc in range(SC):
                o_ps = ops.tile([P, D], F32, tag="o_ps", name=f"ops{sc}")
                for fc in range(DC):
                    for n in range(D // 512):
                        nc.tensor.matmul(
                            o_ps[:, n * 512 : (n + 1) * 512],
                            yT_sb[:, fc, sc * P : (sc + 1) * P],
                            wo_sb[:, fc, n * 512 : (n + 1) * 512],
                            start=(fc == 0), stop=(fc == DC - 1),
                        )
                o_sb = od.tile([P, D], F32, tag="o_sb", name=f"osb{sc}")
                nc.vector.tensor_tensor(o_sb[:], o_ps[:], bor[:], TT.add)
                nc.sync.dma_start(out[sc * P : (sc + 1) * P, :], o_sb[:])

    nc.compile()
    return nc


def _host_prep(inputs):
    import ml_dtypes

    f32 = np.float32
    bf16 = ml_dtypes.bfloat16
    x = np.asarray(inputs["x"], f32)
    bias = np.asarray(inputs["attention_bias"], f32)
    g1 = np.asarray(inputs["g1"], f32)
    b1 = np.asarray(inputs["b1"], f32)
    rr1 = np.asarray(inputs["rrms1"], f32)
    Wq = np.asarray(inputs["Wq"], f32)
    Wk = np.asarray(inputs["Wk"], f32)
    Wv = np.asarray(inputs["Wv"], f32)
    qg = np.asarray(inputs["qg"], f32)
    qb = np.asarray(inputs["qb"], f32)
    kg = np.asarray(inputs["kg"], f32)
    kb = np.asarray(inputs["kb"], f32)
    vg = np.asarray(inputs["vg"], f32)
    vb = np.asarray(inputs["vb"], f32)
    Wo = np.asarray(inputs["Wo"], f32)
    bo = np.asarray(inputs["bo"], f32)
    g2 = np.asarray(inputs["g2"], f32)
    b2 = np.asarray(inputs["b2"], f32)
    rr2 = np.asarray(inputs["rrms2"], f32)

    scale1 = (g1 * (1.0 / np.sqrt(rr1 + EPS_RMS))).astype(f32)
    Wq_e = (Wq * scale1[:, None]).astype(f32)
    Wk_e = (Wk * scale1[:, None]).astype(f32)
    Wv_e = (Wv * scale1[:, None]).astype(f32)
    bq_row = (b1 @ Wq).astype(f32)      # [H*DQ]
    bk_row = (b1 @ Wk).astype(f32)      # [DK]
    bv_row = (b1 @ Wv).astype(f32)      # [DV]
    sc_q = f32(DQ) ** f32(-0.5)
    qg_e = (qg * sc_q).astype(f32)
    qb_e = (qb * sc_q).astype(f32)

    scale2 = (g2 * (1.0 / np.sqrt(rr2 + EPS_RMS))).astype(f32)
    vg_rep = np.tile(vg, H)                      # [H*DV]
    Wo_e = (Wo * vg_rep[:, None] * scale2[None, :]).astype(f32)
    vb_fold = (np.tile(vb, H) @ Wo).astype(f32)  # [D]
    bo_e = ((bo + vb_fold) * scale2 + b2).astype(f32)

    freqs = (
        1.0 / (ROPE_BASE ** (np.arange(HALF, dtype=f32) / HALF))
    ).astype(f32)
    ang = np.arange(S, dtype=f32)[:, None] * freqs[None, :]
    cos = np.cos(ang).astype(f32)                        # [S, 64]
    sin = np.sin(ang).astype(f32)

    c1k = (cos * kg[None, :HALF]).astype(bf16)
    s2nk = (-sin * kg[None, HALF:]).astype(bf16)
    s1k = (sin * kg[None, :HALF]).astype(bf16)
    c2k = (cos * kg[None, HALF:]).astype(bf16)

    rbk_f = np.concatenate(
        [cos * kb[None, :HALF] - sin * kb[None, HALF:],
         sin * kb[None, :HALF] + cos * kb[None, HALF:]], axis=1
    ).astype(f32)
    rbq_f = np.concatenate(
        [cos * qb_e[None, :HALF] - sin * qb_e[None, HALF:],
         sin * qb_e[None, :HALF] + cos * qb_e[None, HALF:]], axis=1
    ).astype(f32)
    has_rbk = bool(np.any(rbk_f))
    has_rbq = bool(np.any(rbq_f))
    has_b = bool(np.any(bq_row) or np.any(bk_row) or np.any(bv_row))

    def dev3(a, n):
        return np.ascontiguousarray(
            a.reshape(n, P, a.shape[-1]).transpose(1, 0, 2)
        )

    rep = lambda v: np.ascontiguousarray(
        np.broadcast_to(v[None, :], (P, v.shape[0]))
    )
    wkv = np.concatenate([Wk_e, Wv_e], axis=1)          # [D, DK+DV]
    shared = {
        "c1k": dev3(c1k, JC),
        "s2nk": dev3(s2nk, JC),
        "s1k": dev3(s1k, JC),
        "c2k": dev3(c2k, JC),
        "wq": dev3(Wq_e.astype(bf16), DC),
        "wkv": dev3(wkv.astype(bf16), DC),
        "wo": dev3(Wo_e.astype(bf16), DC),
        "bor": rep(bo_e),
        "identb": np.eye(P, dtype=bf16),
    }
    if has_b:
        shared["brow"] = np.concatenate(
            [bk_row, bv_row, bq_row]
        ).astype(bf16)[None, :]
    if has_rbk:
        shared["rbk"] = dev3(rbk_f.astype(bf16), JC)

    xdev = []
    for b in range(B):
        xTb = np.ascontiguousarray(x[b].T).astype(bf16)      # [D, S]
        xdev.append(np.ascontiguousarray(
            xTb.reshape(DC, P, JC, P).transpose(1, 2, 0, 3)
        ))
    xTs = [np.ascontiguousarray(x[b].T) for b in range(B)]
    in_maps = []
    for c in range(NCORES):
        b = c // 4
        s0 = (c % 4) * SQ
        m = dict(shared)
        m["xT"] = xdev[b]
        m["xq"] = dev3(xTs[b][:, s0 : s0 + SQ].astype(bf16), DC)
        m["biasT"] = dev3(bias[0, 0, s0 : s0 + SQ, :].T.astype(bf16), JC)
        m["c1q"] = dev3(
            (cos[s0 : s0 + SQ] * qg_e[None, :HALF]).astype(bf16), SC
        )
        m["s2nq"] = dev3(
            (-sin[s0 : s0 + SQ] * qg_e[None, HALF:]).astype(bf16), SC
        )
        m["s1q"] = dev3(
            (sin[s0 : s0 + SQ] * qg_e[None, :HALF]).astype(bf16), SC
        )
        m["c2q"] = dev3(
            (cos[s0 : s0 + SQ] * qg_e[None, HALF:]).astype(bf16), SC
        )
        if has_rbq:
            m["rbq"] = dev3(rbq_f[s0 : s0 + SQ].astype(bf16), SC)
        in_maps.append(m)
    return in_maps, has_rbq, has_rbk, has_b


_NC_CACHE = {}


def _get_nc(has_rbq=False, has_rbk=False, has_b=False):
    key = (has_rbq, has_rbk, has_b)
    if key not in _NC_CACHE:
        _NC_CACHE[key] = build_program(has_rbq, has_rbk, has_b)
    return _NC_CACHE[key]


def kernel(**inputs) -> np.ndarray:
    in_maps, has_rbq, has_rbk, has_b = _host_prep(inputs)
    nc = _get_nc(has_rbq, has_rbk, has_b)
    res = bass_utils.run_bass_kernel_spmd(
        nc, in_maps, core_ids=list(range(NCORES))
    )
    outs = res.results
    full = np.empty((B, S, D), np.float32)
    for c in range(NCORES):
        b = c // 4
        s0 = (c % 4) * SQ
        full[b, s0 : s0 + SQ, :] = outs[c]["out"]
    return full


if __name__ == "__main__":
    nc = _get_nc()
    print("build + compile OK")


# revision 24
# speedup vs baseline: 1.1807x; 1.0165x over previous
"""Trainium2 Bass kernel for an MQA attention block (8 q-heads, shared K/V).

Sharding: 8 cores; core c -> batch b=c//4, query rows s0=(c%4)*512 .. +512,
all 8 heads.  K/V (full sequence, per batch) computed redundantly per core.

v5 design notes:
- The ACT engine (tanh+exp over S*S*H logits, ~2.2us per 2-key-chunk tile)
  is the hard floor; the kernel is built as one long pipeline that keeps
  ACT dense from ~20us onward.
- KV projection row-direct with concatenated [Wk|Wv] moving operand; the
  PSUM accumulator is drained to SBUF bf16 immediately (short bank hold),
  LN stats/normalize run on the SBUF copy, rope is batched 4 chunks at a
  time on DVE in 4x mode.
- Heads 0/1 attention (QK + softcap) is emitted INSIDE the KV loop as
  kT chunks become ready, so the whole KV phase hides under their ACT.
- Q projection row-direct two heads per matmul (N=256) amortizing LDW.
- Attention bias: PE identity-preload (even groups) / DVE add (odd).
- tanh writes an SBUF fp16 intermediate (not in-place PSUM) so the logits
  PSUM bank frees one ACT-pass earlier -> deeper QK pipelining.
- q-LN rstd via Newton rsqrt + q-rope on GPSIMD.
"""

import os
import sys

for _p in ("/opt/trn_rl_repo",):
    if _p not in sys.path and os.path.isdir(_p):
        sys.path.insert(0, _p)

import numpy as np
from contextlib import ExitStack

import concourse.bass as bass
import concourse.mybir as mybir
import concourse.tile as tile
from concourse import bacc
from concourse import bass_utils

F32 = mybir.dt.float32
BF16 = mybir.dt.bfloat16
F16 = mybir.dt.float16

B, S, D = 2, 2048, 1536
H, DQ, DK, DV = 8, 128, 128, 192
P = 128
SQ = S // 4          # 512 query rows per core
DC = D // P          # 12 contraction chunks
JC = S // P          # 16 key chunks
SC = SQ // P         # 4 query-row chunks
NCORES = 8
EPS_RMS = 1e-6
EPS_LN = 1e-5
SOFTCAP = 5.0
ROPE_BASE = 8192.0
HALF = DQ // 2
VW = 256             # vrow inner stride; cols 0:192 v, 192 ones
DKV = DK + DV        # 320

TT = mybir.AluOpType
AF = mybir.ActivationFunctionType


def build_program(has_rbq=False, has_rbk=False, has_b=False):
    nc = bacc.Bacc(
        "TRN2", target_bir_lowering=False, debug=False, num_devices=NCORES
    )

    def din(name, shape, dt=BF16):
        return nc.dram_tensor(name, list(shape), dt, kind="ExternalInput").ap()

    # x in device layout [P, JC, DC, P]: [p, jc, dc, col] = x.T[dc*P+p, jc*P+col]
    xT = din("xT", (P, JC, DC, P))
    xq = din("xq", (P, DC, SQ))          # per-core query-column slice of x.T
    biasT = din("biasT", (P, JC, SQ))
    c1q_t = din("c1q", (P, SC, HALF))
    s2nq_t = din("s2nq", (P, SC, HALF))
    s1q_t = din("s1q", (P, SC, HALF))
    c2q_t = din("c2q", (P, SC, HALF))
    c1k_t = din("c1k", (P, JC, HALF))
    s2nk_t = din("s2nk", (P, JC, HALF))
    s1k_t = din("s1k", (P, JC, HALF))
    c2k_t = din("c2k", (P, JC, HALF))
    wq_t = din("wq", (P, DC, H * DQ))
    wkv_t = din("wkv", (P, DC, DKV))     # [Wk | Wv] concatenated
    wo_t = din("wo", (P, DC, D))
    bor_t = din("bor", (P, D), F32)
    identb_t = din("identb", (P, P))
    if has_b:
        brow_t = din("brow", (1, DKV + H * DQ))
    if has_rbq:
        rbq_t = din("rbq", (P, SC, DQ))
    if has_rbk:
        rbk_t = din("rbk", (P, JC, DK))
    out = nc.dram_tensor("out", [SQ, D], F32, kind="ExternalOutput").ap()
    DEBUG = os.environ.get("KDEBUG", "0") == "1"
    if DEBUG:
        dbg_kT = nc.dram_tensor(
            "dbg_kT", [P, S], BF16, kind="ExternalOutput").ap()
        dbg_vrow = nc.dram_tensor(
            "dbg_vrow", [P, JC, VW], BF16, kind="ExternalOutput").ap()
        dbg_qT0 = nc.dram_tensor(
            "dbg_qT0", [P, SQ], BF16, kind="ExternalOutput").ap()
        dbg_pts0 = nc.dram_tensor(
            "dbg_pts0", [P, JC, SQ], BF16, kind="ExternalOutput").ap()
        dbg_yp0 = nc.dram_tensor(
            "dbg_yp0", [P, SC, 2 * DV], BF16, kind="ExternalOutput").ap()

    with tile.TileContext(nc) as tc, ExitStack() as ctx:
        const = ctx.enter_context(tc.tile_pool(name="const", bufs=1))
        persist = ctx.enter_context(tc.tile_pool(name="persist", bufs=1))
        qt = ctx.enter_context(tc.tile_pool(name="qt", bufs=2))
        att = ctx.enter_context(tc.tile_pool(name="att", bufs=2))
        qps_cm = tc.tile_pool(name="qps", bufs=1, space="PSUM")
        qpsp = qps_cm.__enter__()
        scr_cm = tc.tile_pool(name="scr", bufs=1, space="PSUM")
        scr = scr_cm.__enter__()
        apq_cm = tc.tile_pool(name="apq", bufs=2, space="PSUM")
        apq = apq_cm.__enter__()

        # ---------------- constants (DMA emission order matters) ----------
        identb = const.tile([P, P], BF16)
        nc.sync.dma_start(identb[:], identb_t)
        wkv_sb = const.tile([P, DC, DKV], BF16)
        nc.sync.dma_start(wkv_sb[:], wkv_t)
        xq_sb = persist.tile([P, DC, SQ], BF16)
        wq_sb = persist.tile([P, DC, 4 * DQ], BF16)
        biasT_sb = persist.tile([P, JC, SQ], BF16)

        eps_sb = const.tile([P, 1], F32)
        nc.vector.memset(eps_sb[:], EPS_LN)
        if has_b:
            brow = const.tile([1, DKV + H * DQ], BF16)
            nc.sync.dma_start(brow[:], brow_t)
            ones1 = const.tile([1, P], BF16)
            nc.vector.memset(ones1[:], 1.0)

        def load_tab(t, n, nm):
            tt = const.tile([P, n, HALF], BF16, tag=nm, name=nm)
            nc.sync.dma_start(tt[:], t)
            return tt

        c1k = load_tab(c1k_t, JC, "c1k")
        s2nk = load_tab(s2nk_t, JC, "s2nk")
        s1k = load_tab(s1k_t, JC, "s1k")
        c2k = load_tab(c2k_t, JC, "c2k")
        c1q = load_tab(c1q_t, SC, "c1q")
        s2nq = load_tab(s2nq_t, SC, "s2nq")
        s1q = load_tab(s1q_t, SC, "s1q")
        c2q = load_tab(c2q_t, SC, "c2q")
        if has_rbk:
            rbk = const.tile([P, JC, DK], BF16)
            nc.sync.dma_start(rbk[:], rbk_t)
        if has_rbq:
            rbq = const.tile([P, SC, DQ], BF16)
            nc.sync.dma_start(rbq[:], rbq_t)

        # ---------------- persistent activations ----------------
        kT_sb = persist.tile([P, S], BF16)          # rope'd k, [dk, s]
        vrow_sb = persist.tile([P, JC, VW], BF16)   # v rows + ones col
        nc.vector.memset(vrow_sb[:, :, DV : DV + 1], 1.0)
        qT = [
            persist.tile([P, SQ], BF16, tag=f"q{h}", name=f"qT{h}")
            for h in range(H)
        ]
        yp = [
            persist.tile([P, SC, 2 * DV], BF16, tag=f"yp{p}", name=f"yp{p}")
            for p in range(4)
        ]
        yT_sb = persist.tile([P, DC, SQ], BF16)

        g = (nc.gpsimd if os.environ.get("USE_GPSIMD", "1") == "1"
             else nc.vector)
        pts = {}

        # ---------------- attention primitives ----------------
        def qk_group(h, jg):
            pq = apq.tile([P, 2, SQ], F32, tag="pq", name=f"pq{h}{jg}")
            dve_bias = jg % 4 == 3
            for c in range(2):
                jc = jg * 2 + c
                if not dve_bias:
                    nc.tensor.matmul(
                        pq[:, c, :], identb[:], biasT_sb[:, jc, :],
                        start=True, stop=False,
                    )
                nc.tensor.matmul(
                    pq[:, c, :],
                    kT_sb[:, jc * P : (jc + 1) * P], qT[h][:],
                    start=dve_bias, stop=True,
                )
            if dve_bias:
                nc.vector.tensor_tensor(
                    pq[:], pq[:], biasT_sb[:, jg * 2 : jg * 2 + 2, :],
                    TT.add,
                )
            t16 = att.tile([P, 2, SQ], F16, tag="t16", name=f"t16_{h}{jg}")
            nc.scalar.activation(
                t16[:], pq[:], AF.Tanh, scale=1.0 / SOFTCAP
            )
            nc.scalar.activation(
                pts[h][:, jg * 2 : jg * 2 + 2, :], t16[:],
                AF.Exp, scale=SOFTCAP,
            )

        # -------- q pipeline (row-direct, two heads per matmul) --------
        def stage_q_proj(hp, t, parts):
            if t == 0:
                parts.append(
                    qpsp.tile([P, SC, 2, DQ], F32, tag="q_ps",
                              name=f"qps{hp}")
                )
            q_ps = parts[0]
            last = DC - 1
            h0 = 2 * hp
            w0 = (h0 % 4) * DQ
            for dc in range(DC):
                nc.tensor.matmul(
                    q_ps[:, t, :, :],
                    xq_sb[:, dc, t * P : (t + 1) * P],
                    wq_sb[:, dc, w0 : w0 + 2 * DQ],
                    start=(dc == 0), stop=(dc == last) and not has_b,
                )
            if has_b:
                nc.tensor.matmul(
                    q_ps[:, t, :, :], ones1[:],
                    brow[:, DKV + h0 * DQ : DKV + (h0 + 2) * DQ],
                    start=False, stop=True,
                )
            st6 = qt.tile([P, 2, 6], F32, tag=f"qst{t}", name=f"qst{hp}_{t}")
            for j in range(2):
                nc.vector.bn_stats(st6[:, j, :], q_ps[:, t, j, :])
            parts.append(st6)

        def stage_q_ln(hp, parts):
            q_ps = parts[0]
            qag = qt.tile([P, SC, 2, 2], F32, tag="qag", name=f"qag{hp}")
            for t in range(SC):
                for j in range(2):
                    nc.vector.bn_aggr(qag[:, t, j, :], parts[1 + t][:, j, :])
            qv = qt.tile([P, SC, 2, 1], F32, tag="qv", name=f"qv{hp}")
            qy = qt.tile([P, SC, 2, 1], F32, tag="qy", name=f"qy{hp}")
            qw2 = qt.tile([P, SC, 2, 1], F32, tag="qw2", name=f"qw2{hp}")
            g.tensor_scalar(qv[:], qag[:, :, :, 1:2], EPS_LN, None, TT.add)
            g.tensor_scalar(qy[:], qv[:], -0.5, 1.5, TT.mult, TT.add)
            for _ in range(3):
                g.tensor_tensor(qw2[:], qy[:], qy[:], TT.mult)
                g.tensor_tensor(qw2[:], qw2[:], qv[:], TT.mult)
                g.tensor_scalar(qw2[:], qw2[:], -0.5, 1.5, TT.mult, TT.add)
                g.tensor_tensor(qy[:], qy[:], qw2[:], TT.mult)
            xnq = qt.tile([P, SC, 2, DQ], BF16, tag="xnq", name=f"xnq{hp}")
            for t in range(SC):
                for j in range(2):
                    nc.vector.tensor_scalar(
                        xnq[:, t, j, :], q_ps[:, t, j, :],
                        qag[:, t, j, 0:1], qy[:, t, j, 0:1],
                        TT.subtract, TT.mult,
                    )
            for j in range(2):
                h = 2 * hp + j
                qz1 = qt.tile([P, SC, HALF], BF16, tag="qz1", name=f"qz1{h}")
                qz2 = qt.tile([P, SC, HALF], BF16, tag="qz2", name=f"qz2{h}")
                qr = qt.tile([P, SC, DQ], BF16, tag="qr", name=f"qr{h}")
                xj = xnq[:, :, j, :]
                g.tensor_tensor(qz1[:], xj[:, :, :HALF], c1q[:], TT.mult)
                g.tensor_tensor(qz2[:], xj[:, :, HALF:], s2nq[:], TT.mult)
                g.tensor_tensor(qr[:, :, :HALF], qz1[:], qz2[:], TT.add)
                g.tensor_tensor(qz1[:], xj[:, :, :HALF], s1q[:], TT.mult)
                g.tensor_tensor(qz2[:], xj[:, :, HALF:], c2q[:], TT.mult)
                g.tensor_tensor(qr[:, :, HALF:], qz1[:], qz2[:], TT.add)
                if has_rbq:
                    g.tensor_tensor(qr[:], qr[:], rbq[:], TT.add)
                for t in range(SC):
                    sct = scr.tile([P, P], BF16, tag="scb", name=f"qbt{h}{t}")
                    nc.tensor.transpose(sct[:], qr[:, t, :], identb[:])
                    nc.vector.tensor_copy(
                        qT[h][:, t * P : (t + 1) * P], sct[:]
                    )

        # =========================================================
        # KV loop with heads 0/1 attention overlapped
        # =========================================================
        for h in (0, 1):
            pts[h] = att.tile([P, JC, SQ], BF16, tag="pt", name=f"pt{h}")

        with (
            tc.tile_pool(name="kvx", bufs=3) as kvx,
            tc.tile_pool(name="kvt", bufs=2) as kvt,
            tc.tile_pool(name="kvrp", bufs=5) as kvrp,
            tc.tile_pool(name="kvps", bufs=1, space="PSUM") as kvps,
        ):
            parts01 = []
            parts1 = []  # pair 1 = heads 2,3

            xts = {}

            def xt_fetch(jc):
                if jc < JC and jc not in xts:
                    xt = kvx.tile([P, DC, P], BF16, tag="xt",
                                  name=f"xt{jc}")
                    nc.sync.dma_start(xt[:], xT[:, jc, :, :])
                    xts[jc] = xt

            def kv_chunk(jc):
                xt_fetch(jc)
                xt_fetch(jc + 1)
                xt_fetch(jc + 2)
                xt = xts.pop(jc)
                if jc == 0:
                    nc.sync.dma_start(xq_sb[:], xq)
                    nc.sync.dma_start(wq_sb[:], wq_t[:, :, 0 : 4 * DQ])
                if jc == 2:
                    nc.sync.dma_start(biasT_sb[:], biasT)
                kv_ps = kvps.tile([P, DKV], F32, tag="kv_ps",
                                  name=f"kvps{jc}")
                last = DC - 1
                for dc in range(DC):
                    nc.tensor.matmul(
                        kv_ps[:], xt[:, dc, :], wkv_sb[:, dc, :],
                        start=(dc == 0), stop=(dc == last) and not has_b,
                    )
                if has_b:
                    nc.tensor.matmul(
                        kv_ps[:], ones1[:], brow[:, 0:DKV],
                        start=False, stop=True,
                    )
                # drain to SBUF immediately (frees the PSUM bank)
                kvr = kvrp.tile([P, DKV], BF16, tag="kvr", name=f"kvr{jc}")
                nc.vector.tensor_copy(kvr[:], kv_ps[:])
                kv_chunk.kvrs[jc % 4] = kvr
                # stats on the bf16 copy; (mean, var) pairs into batch tile
                if jc % 4 == 0:
                    kv_chunk.agg4 = kvt.tile(
                        [P, 4, 2, 2], F32, tag="agg4", name=f"agg4_{jc // 4}"
                    )
                kst = kvt.tile([P, 6], F32, tag="kst", name=f"kst{jc}")
                nc.vector.bn_stats(kst[:], kvr[:, 0:DK])
                nc.vector.bn_aggr(kv_chunk.agg4[:, jc % 4, 0, :], kst[:])
                vst = kvt.tile([P, 6], F32, tag="vst", name=f"vst{jc}")
                nc.vector.bn_stats(vst[:], kvr[:, DK:DKV])
                nc.vector.bn_aggr(kv_chunk.agg4[:, jc % 4, 1, :], vst[:])

            kv_chunk.kvrs = [None] * 4

            def rope_batch(bi):
                # newton rsqrt for the 4 chunks' k/v (no ACT table switch),
                # then normalize and rope on DVE (bf16 4x mode)
                agg4 = kv_chunk.agg4
                nv = kvt.tile([P, 4, 2, 1], F32, tag="nv", name=f"nv{bi}")
                ny = kvt.tile([P, 4, 2, 1], F32, tag="ny", name=f"ny{bi}")
                nw = kvt.tile([P, 4, 2, 1], F32, tag="nw", name=f"nw{bi}")
                g.tensor_scalar(nv[:], agg4[:, :, :, 1:2], EPS_LN, None,
                                TT.add)
                g.tensor_scalar(ny[:], nv[:], -0.5, 1.5, TT.mult, TT.add)
                for _ in range(3):
                    g.tensor_tensor(nw[:], ny[:], ny[:], TT.mult)
                    g.tensor_tensor(nw[:], nw[:], nv[:], TT.mult)
                    g.tensor_scalar(nw[:], nw[:], -0.5, 1.5, TT.mult, TT.add)
                    g.tensor_tensor(ny[:], ny[:], nw[:], TT.mult)
                xnk4 = kvt.tile([P, 4, DK], BF16, tag="xnk4", name=f"xnk4_{bi}")
                for t in range(4):
                    jc = bi * 4 + t
                    kvr = kv_chunk.kvrs[t]
                    nc.vector.tensor_scalar(
                        xnk4[:, t, :], kvr[:, 0:DK],
                        agg4[:, t, 0, 0:1], ny[:, t, 0, 0:1],
                        TT.subtract, TT.mult,
                    )
                    nc.vector.tensor_scalar(
                        vrow_sb[:, jc, 0:DV], kvr[:, DK:DKV],
                        agg4[:, t, 1, 0:1], ny[:, t, 1, 0:1],
                        TT.subtract, TT.mult,
                    )
                jcs = slice(bi * 4, bi * 4 + 4)
                kz1 = kvt.tile([P, 4, HALF], BF16, tag="kz1", name=f"kz1{bi}")
                kz2 = kvt.tile([P, 4, HALF], BF16, tag="kz2", name=f"kz2{bi}")
                kr4 = kvt.tile([P, 4, DK], BF16, tag="kr4", name=f"kr4{bi}")
                V = nc.vector
                V.tensor_tensor(kz1[:], xnk4[:, :, :HALF], c1k[:, jcs, :],
                                TT.mult)
                V.tensor_tensor(kz2[:], xnk4[:, :, HALF:], s2nk[:, jcs, :],
                                TT.mult)
                V.tensor_tensor(kr4[:, :, :HALF], kz1[:], kz2[:], TT.add)
                V.tensor_tensor(kz1[:], xnk4[:, :, :HALF], s1k[:, jcs, :],
                                TT.mult)
                V.tensor_tensor(kz2[:], xnk4[:, :, HALF:], c2k[:, jcs, :],
                                TT.mult)
                V.tensor_tensor(kr4[:, :, HALF:], kz1[:], kz2[:], TT.add)
                if has_rbk:
                    V.tensor_tensor(kr4[:], kr4[:], rbk[:, jcs, :], TT.add)
                for t in range(4):
                    jc = bi * 4 + t
                    scb = scr.tile([P, P], BF16, tag="scb", name=f"kbt{jc}")
                    nc.tensor.transpose(scb[:], kr4[:, t, :], identb[:])
                    nc.vector.tensor_copy(
                        kT_sb[:, jc * P : (jc + 1) * P], scb[:]
                    )

            # schedule: kv chunks, rope batches, stageQ pair0/pair1,
            # heads 0/1 qk groups as kT becomes available
            for jc in range(4):
                kv_chunk(jc)
            rope_batch(0)
            for t in range(SC):
                stage_q_proj(0, t, parts01)
            stage_q_ln(0, parts01)
            for jc in range(4, 8):
                kv_chunk(jc)
                if jc == 5:
                    qk_group(0, 0)
                    qk_group(0, 1)
                if jc == 6:
                    qk_group(1, 0)
                if jc == 7:
                    qk_group(1, 1)
            rope_batch(1)
            for jc in range(8, 12):
                kv_chunk(jc)
                if jc == 8:
                    qk_group(0, 2)
                if jc == 9:
                    qk_group(0, 3)
                    stage_q_proj(1, 0, parts1)
                if jc == 10:
                    qk_group(1, 2)
                    stage_q_proj(1, 1, parts1)
                    wo_sb = persist.tile([P, DC, D], BF16)
                    nc.sync.dma_start(wo_sb[:], wo_t)
                    bor = persist.tile([P, D], F32)
                    nc.sync.dma_start(bor[:], bor_t)
                if jc == 11:
                    qk_group(1, 3)
                    stage_q_proj(1, 2, parts1)
            rope_batch(2)
            for jc in range(12, 16):
                kv_chunk(jc)
                if jc == 12:
                    qk_group(0, 4)
                if jc == 13:
                    qk_group(0, 5)
                    stage_q_proj(1, 3, parts1)
                    nc.sync.dma_start(
                        wq_sb[:], wq_t[:, :, 4 * DQ : 8 * DQ]
                    )
                if jc == 14:
                    qk_group(1, 4)
                if jc == 15:
                    qk_group(1, 5)
                    stage_q_ln(1, parts1)
            rope_batch(3)
            for jg in (6, 7):
                qk_group(0, jg)
            for jg in (6, 7):
                qk_group(1, jg)

        # =========================================================
        # heads 2..7 + PV pipeline
        # =========================================================
        with (
            tc.tile_pool(name="ay", bufs=1, space="PSUM") as ay,
        ):
            def pv_chunk(h, ic):
                y_ps = ay.tile([P, DV + 1], F32, tag="y_ps",
                               name=f"yps{h}{ic}")
                for jc in range(JC):
                    nc.tensor.matmul(
                        y_ps[:],
                        pts[h][:, jc, ic * P : (ic + 1) * P],
                        vrow_sb[:, jc, : DV + 1],
                        start=(jc == 0), stop=(jc == JC - 1),
                    )
                rcp = att.tile([P, 1], F32, tag="rcp", name=f"rcp{h}{ic}")
                nc.vector.reciprocal(rcp[:], y_ps[:, DV : DV + 1])
                nc.vector.tensor_scalar_mul(
                    yp[h // 2][:, ic, (h % 2) * DV : (h % 2 + 1) * DV],
                    y_ps[:, :DV], rcp[:, 0:1],
                )

            def oproj_transposes(p):
                for sc in range(SC):
                    for fcl in range(3):
                        fc = 3 * p + fcl
                        scb = scr.tile([P, P], BF16, tag="scb",
                                       name=f"so{p}{sc}{fcl}")
                        nc.tensor.transpose(
                            scb[:],
                            yp[p][:, sc, fcl * P : (fcl + 1) * P],
                            identb[:],
                        )
                        nc.vector.tensor_copy(
                            yT_sb[:, fc, sc * P : (sc + 1) * P], scb[:]
                        )

            # pv(0) immediately after the overlap section so the pts[0]
            # buffer (reused by pts[2]) frees before head 2's exps
            for ic in range(SC):
                pv_chunk(0, ic)

            parts_by_pair = {}
            for h in range(2, H):
                pts[h] = att.tile([P, JC, SQ], BF16, tag="pt", name=f"pt{h}")
                fillers = []
                hp = h // 2 + 1
                if hp < 4:
                    if h % 2 == 0:
                        partsn = []
                        parts_by_pair[hp] = partsn
                        for t in range(SC):
                            fillers.append(
                                lambda hp=hp, t=t, pn=partsn: stage_q_proj(
                                    hp, t, pn
                                )
                            )
                    else:
                        fillers.append(
                            lambda hp=hp: stage_q_ln(hp, parts_by_pair[hp])
                        )
                # pv of the previous head must fully emit during this head
                # (pts pool has 2 buffers)
                for ic in range(SC):
                    fillers.append(lambda h=h, ic=ic: pv_chunk(h - 1, ic))
                if 4 <= h <= 6:
                    fillers.append(lambda p=h - 4: oproj_transposes(p))

                nf = len(fillers)
                done = 0
                for jg in range(JC // 2):
                    qk_group(h, jg)
                    want = (nf * (jg + 1) + 7) // 8
                    while done < want:
                        fillers[done]()
                        done += 1
            # tail
            for ic in range(SC):
                pv_chunk(H - 1, ic)
            oproj_transposes(3)
            if DEBUG:
                nc.sync.dma_start(dbg_kT[:], kT_sb[:])
                nc.sync.dma_start(dbg_vrow[:], vrow_sb[:])
                nc.sync.dma_start(dbg_qT0[:], qT[0][:])
                nc.sync.dma_start(dbg_pts0[:], pts[0][:])
                nc.sync.dma_start(dbg_yp0[:], yp[0][:])

        apq_cm.__exit__(None, None, None)
        scr_cm.__exit__(None, None, None)
        qps_cm.__exit__(None, None, None)

        # =========================================================
        # Output projection (bf16)
        # =========================================================
        with (
            tc.tile_pool(name="od", bufs=2) as od,
            tc.tile_pool(name="ops", bufs=2, space="PSUM") as ops,
        ):
            for sc in range(SC):
                o_ps = ops.tile([P, D], F32, tag="o_ps", name=f"ops{sc}")
                for fc in range(DC):
                    for n in range(D // 512):
                        nc.tensor.matmul(
                            o_ps[:, n * 512 : (n + 1) * 512],
                            yT_sb[:, fc, sc * P : (sc + 1) * P],
                            wo_sb[:, fc, n * 512 : (n + 1) * 512],
                            start=(fc == 0), stop=(fc == DC - 1),
                        )
                o_sb = od.tile([P, D], F32, tag="o_sb", name=f"osb{sc}")
                nc.vector.tensor_tensor(o_sb[:], o_ps[:], bor[:], TT.add)
                nc.sync.dma_start(out[sc * P : (sc + 1) * P, :], o_sb[:])

    nc.compile()
    return nc


def _host_prep(inputs):
    import ml_dtypes

    f32 = np.float32
    bf16 = ml_dtypes.bfloat16
    x = np.asarray(inputs["x"], f32)
    bias = np.asarray(inputs["attention_bias"], f32)
    g1 = np.asarray(inputs["g1"], f32)
    b1 = np.asarray(inputs["b1"], f32)
    rr1 = np.asarray(inputs["rrms1"], f32)
    Wq = np.asarray(inputs["Wq"], f32)
    Wk = np.asarray(inputs["Wk"], f32)
    Wv = np.asarray(inputs["Wv"], f32)
    qg = np.asarray(inputs["qg"], f32)
    qb = np.asarray(inputs["qb"], f32)
    kg = np.asarray(inputs["kg"], f32)
    kb = np.asarray(inputs["kb"], f32)
    vg = np.asarray(inputs["vg"], f32)
    vb = np.asarray(inputs["vb"], f32)
    Wo = np.asarray(inputs["Wo"], f32)
    bo = np.asarray(inputs["bo"], f32)
    g2 = np.asarray(inputs["g2"], f32)
    b2 = np.asarray(inputs["b2"], f32)
    rr2 = np.asarray(inputs["rrms2"], f32)

    scale1 = (g1 * (1.0 / np.sqrt(rr1 + EPS_RMS))).astype(f32)
    Wq_e = (Wq * scale1[:, None]).astype(f32)
    Wk_e = (Wk * scale1[:, None]).astype(f32)
    Wv_e = (Wv * scale1[:, None]).astype(f32)
    bq_row = (b1 @ Wq).astype(f32)      # [H*DQ]
    bk_row = (b1 @ Wk).astype(f32)      # [DK]
    bv_row = (b1 @ Wv).astype(f32)      # [DV]
    sc_q = f32(DQ) ** f32(-0.5)
    qg_e = (qg * sc_q).astype(f32)
    qb_e = (qb * sc_q).astype(f32)

    scale2 = (g2 * (1.0 / np.sqrt(rr2 + EPS_RMS))).astype(f32)
    vg_rep = np.tile(vg, H)                      # [H*DV]
    Wo_e = (Wo * vg_rep[:, None] * scale2[None, :]).astype(f32)
    vb_fold = (np.tile(vb, H) @ Wo).astype(f32)  # [D]
    bo_e = ((bo + vb_fold) * scale2 + b2).astype(f32)

    freqs = (
        1.0 / (ROPE_BASE ** (np.arange(HALF, dtype=f32) / HALF))
    ).astype(f32)
    ang = np.arange(S, dtype=f32)[:, None] * freqs[None, :]
    cos = np.cos(ang).astype(f32)                        # [S, 64]
    sin = np.sin(ang).astype(f32)

    c1k = (cos * kg[None, :HALF]).astype(bf16)
    s2nk = (-sin * kg[None, HALF:]).astype(bf16)
    s1k = (sin * kg[None, :HALF]).astype(bf16)
    c2k = (cos * kg[None, HALF:]).astype(bf16)

    rbk_f = np.concatenate(
        [cos * kb[None, :HALF] - sin * kb[None, HALF:],
         sin * kb[None, :HALF] + cos * kb[None, HALF:]], axis=1
    ).astype(f32)
    rbq_f = np.concatenate(
        [cos * qb_e[None, :HALF] - sin * qb_e[None, HALF:],
         sin * qb_e[None, :HALF] + cos * qb_e[None, HALF:]], axis=1
    ).astype(f32)
    has_rbk = bool(np.any(rbk_f))
    has_rbq = bool(np.any(rbq_f))
    has_b = bool(np.any(bq_row) or np.any(bk_row) or np.any(bv_row))

    def dev3(a, n):
        return np.ascontiguousarray(
            a.reshape(n, P, a.shape[-1]).transpose(1, 0, 2)
        )

    rep = lambda v: np.ascontiguousarray(
        np.broadcast_to(v[None, :], (P, v.shape[0]))
    )
    wkv = np.concatenate([Wk_e, Wv_e], axis=1)          # [D, DK+DV]
    shared = {
        "c1k": dev3(c1k, JC),
        "s2nk": dev3(s2nk, JC),
        "s1k": dev3(s1k, JC),
        "c2k": dev3(c2k, JC),
        "wq": dev3(Wq_e.astype(bf16), DC),
        "wkv": dev3(wkv.astype(bf16), DC),
        "wo": dev3(Wo_e.astype(bf16), DC),
        "bor": rep(bo_e),
        "identb": np.eye(P, dtype=bf16),
    }
    if has_b:
        shared["brow"] = np.concatenate(
            [bk_row, bv_row, bq_row]
        ).astype(bf16)[None, :]
    if has_rbk:
        shared["rbk"] = dev3(rbk_f.astype(bf16), JC)

    xdev = []
    for b in range(B):
        xTb = np.ascontiguousarray(x[b].T).astype(bf16)      # [D, S]
        xdev.append(np.ascontiguousarray(
            xTb.reshape(DC, P, JC, P).transpose(1, 2, 0, 3)
        ))
    xTs = [np.ascontiguousarray(x[b].T) for b in range(B)]
    in_maps = []
    for c in range(NCORES):
        b = c // 4
        s0 = (c % 4) * SQ
        m = dict(shared)
        m["xT"] = xdev[b]
        m["xq"] = dev3(xTs[b][:, s0 : s0 + SQ].astype(bf16), DC)
        m["biasT"] = dev3(bias[0, 0, s0 : s0 + SQ, :].T.astype(bf16), JC)
        m["c1q"] = dev3(
            (cos[s0 : s0 + SQ] * qg_e[None, :HALF]).astype(bf16), SC
        )
        m["s2nq"] = dev3(
            (-sin[s0 : s0 + SQ] * qg_e[None, HALF:]).astype(bf16), SC
        )
        m["s1q"] = dev3(
            (sin[s0 : s0 + SQ] * qg_e[None, :HALF]).astype(bf16), SC
        )
        m["c2q"] = dev3(
            (cos[s0 : s0 + SQ] * qg_e[None, HALF:]).astype(bf16), SC
        )
        if has_rbq:
            m["rbq"] = dev3(rbq_f[s0 : s0 + SQ].astype(bf16), SC)
        in_maps.append(m)
    return in_maps, has_rbq, has_rbk, has_b


_NC_CACHE = {}


def _get_nc(has_rbq=False, has_rbk=False, has_b=False):
    key = (has_rbq, has_rbk, has_b)
    if key not in _NC_CACHE:
        _NC_CACHE[key] = build_program(has_rbq, has_rbk, has_b)
    return _NC_CACHE[key]


def kernel(**inputs) -> np.ndarray:
    in_maps, has_rbq, has_rbk, has_b = _host_prep(inputs)
    nc = _get_nc(has_rbq, has_rbk, has_b)
    res = bass_utils.run_bass_kernel_spmd(
        nc, in_maps, core_ids=list(range(NCORES))
    )
    outs = res.results
    full = np.empty((B, S, D), np.float32)
    for c in range(NCORES):
        b = c // 4
        s0 = (c % 4) * SQ
        full[b, s0 : s0 + SQ, :] = outs[c]["out"]
    return full


if __name__ == "__main__":
    nc = _get_nc()
    print("build + compile OK")


# revision 25
# speedup vs baseline: 1.2127x; 1.0271x over previous
"""Trainium2 Bass kernel for an MQA attention block (8 q-heads, shared K/V).

Sharding: 8 cores; core c -> batch b=c//4, query rows s0=(c%4)*512 .. +512,
all 8 heads.  K/V (full sequence, per batch) computed redundantly per core.

v5 design notes:
- The ACT engine (tanh+exp over S*S*H logits, ~2.2us per 2-key-chunk tile)
  is the hard floor; the kernel is built as one long pipeline that keeps
  ACT dense from ~20us onward.
- KV projection row-direct with concatenated [Wk|Wv] moving operand; the
  PSUM accumulator is drained to SBUF bf16 immediately (short bank hold),
  LN stats/normalize run on the SBUF copy, rope is batched 4 chunks at a
  time on DVE in 4x mode.
- Heads 0/1 attention (QK + softcap) is emitted INSIDE the KV loop as
  kT chunks become ready, so the whole KV phase hides under their ACT.
- Q projection row-direct two heads per matmul (N=256) amortizing LDW.
- Attention bias: PE identity-preload (even groups) / DVE add (odd).
- tanh writes an SBUF fp16 intermediate (not in-place PSUM) so the logits
  PSUM bank frees one ACT-pass earlier -> deeper QK pipelining.
- q-LN rstd via Newton rsqrt + q-rope on GPSIMD.
"""

import os
import sys

for _p in ("/opt/trn_rl_repo",):
    if _p not in sys.path and os.path.isdir(_p):
        sys.path.insert(0, _p)

import numpy as np
from contextlib import ExitStack

import concourse.bass as bass
import concourse.mybir as mybir
import concourse.tile as tile
from concourse import bacc
from concourse import bass_utils

F32 = mybir.dt.float32
BF16 = mybir.dt.bfloat16
F16 = mybir.dt.float16

B, S, D = 2, 2048, 1536
H, DQ, DK, DV = 8, 128, 128, 192
P = 128
SQ = S // 4          # 512 query rows per core
DC = D // P          # 12 contraction chunks
JC = S // P          # 16 key chunks
SC = SQ // P         # 4 query-row chunks
NCORES = 8
EPS_RMS = 1e-6
EPS_LN = 1e-5
SOFTCAP = 5.0
ROPE_BASE = 8192.0
HALF = DQ // 2
VW = 256             # vrow inner stride; cols 0:192 v, 192 ones
DKV = DK + DV        # 320

TT = mybir.AluOpType
AF = mybir.ActivationFunctionType


def build_program(has_rbq=False, has_rbk=False, has_b=False):
    nc = bacc.Bacc(
        "TRN2", target_bir_lowering=False, debug=False, num_devices=NCORES
    )

    def din(name, shape, dt=BF16):
        return nc.dram_tensor(name, list(shape), dt, kind="ExternalInput").ap()

    # x in device layout [P, JC, DC, P]: [p, jc, dc, col] = x.T[dc*P+p, jc*P+col]
    xT = din("xT", (P, JC, DC, P))
    xq = din("xq", (P, DC, SQ))          # per-core query-column slice of x.T
    biasT = din("biasT", (P, JC, SQ))
    c1q_t = din("c1q", (P, SC, HALF))
    s2nq_t = din("s2nq", (P, SC, HALF))
    s1q_t = din("s1q", (P, SC, HALF))
    c2q_t = din("c2q", (P, SC, HALF))
    c1k_t = din("c1k", (P, JC, HALF))
    s2nk_t = din("s2nk", (P, JC, HALF))
    s1k_t = din("s1k", (P, JC, HALF))
    c2k_t = din("c2k", (P, JC, HALF))
    wq_t = din("wq", (P, DC, H * DQ))
    wkv_t = din("wkv", (P, DC, DKV))     # [Wk | Wv] concatenated
    wo_t = din("wo", (P, DC, D))
    bor_t = din("bor", (P, D), F32)
    identb_t = din("identb", (P, P))
    if has_b:
        brow_t = din("brow", (1, DKV + H * DQ))
    if has_rbq:
        rbq_t = din("rbq", (P, SC, DQ))
    if has_rbk:
        rbk_t = din("rbk", (P, JC, DK))
    out = nc.dram_tensor("out", [SQ, D], F32, kind="ExternalOutput").ap()
    DEBUG = os.environ.get("KDEBUG", "0") == "1"
    if DEBUG:
        dbg_kT = nc.dram_tensor(
            "dbg_kT", [P, S], BF16, kind="ExternalOutput").ap()
        dbg_vrow = nc.dram_tensor(
            "dbg_vrow", [P, JC, VW], BF16, kind="ExternalOutput").ap()
        dbg_qT0 = nc.dram_tensor(
            "dbg_qT0", [P, SQ], BF16, kind="ExternalOutput").ap()
        dbg_pts0 = nc.dram_tensor(
            "dbg_pts0", [P, JC, SQ], BF16, kind="ExternalOutput").ap()
        dbg_yp0 = nc.dram_tensor(
            "dbg_yp0", [P, SC, 2 * DV], BF16, kind="ExternalOutput").ap()

    with tile.TileContext(nc) as tc, ExitStack() as ctx:
        const = ctx.enter_context(tc.tile_pool(name="const", bufs=1))
        persist = ctx.enter_context(tc.tile_pool(name="persist", bufs=1))
        qt = ctx.enter_context(tc.tile_pool(name="qt", bufs=2))
        att = ctx.enter_context(tc.tile_pool(name="att", bufs=2))
        qps_cm = tc.tile_pool(name="qps", bufs=1, space="PSUM")
        qpsp = qps_cm.__enter__()
        scr_cm = tc.tile_pool(name="scr", bufs=1, space="PSUM")
        scr = scr_cm.__enter__()
        apq_cm = tc.tile_pool(name="apq", bufs=2, space="PSUM")
        apq = apq_cm.__enter__()

        # ---------------- constants (DMA emission order matters) ----------
        identb = const.tile([P, P], BF16)
        nc.sync.dma_start(identb[:], identb_t)
        wkv_sb = const.tile([P, DC, DKV], BF16)
        nc.sync.dma_start(wkv_sb[:], wkv_t)
        xq_sb = persist.tile([P, DC, SQ], BF16)
        wq_sb = persist.tile([P, DC, 4 * DQ], BF16)
        biasT_sb = persist.tile([P, JC, SQ], BF16)

        eps_sb = const.tile([P, 1], F32)
        nc.vector.memset(eps_sb[:], EPS_LN)
        if has_b:
            brow = const.tile([1, DKV + H * DQ], BF16)
            nc.sync.dma_start(brow[:], brow_t)
            ones1 = const.tile([1, P], BF16)
            nc.vector.memset(ones1[:], 1.0)

        def load_tab(t, n, nm):
            tt = const.tile([P, n, HALF], BF16, tag=nm, name=nm)
            nc.sync.dma_start(tt[:], t)
            return tt

        c1k = load_tab(c1k_t, JC, "c1k")
        s2nk = load_tab(s2nk_t, JC, "s2nk")
        s1k = load_tab(s1k_t, JC, "s1k")
        c2k = load_tab(c2k_t, JC, "c2k")
        c1q = load_tab(c1q_t, SC, "c1q")
        s2nq = load_tab(s2nq_t, SC, "s2nq")
        s1q = load_tab(s1q_t, SC, "s1q")
        c2q = load_tab(c2q_t, SC, "c2q")
        if has_rbk:
            rbk = const.tile([P, JC, DK], BF16)
            nc.sync.dma_start(rbk[:], rbk_t)
        if has_rbq:
            rbq = const.tile([P, SC, DQ], BF16)
            nc.sync.dma_start(rbq[:], rbq_t)

        # ---------------- persistent activations ----------------
        kT_sb = persist.tile([P, S], BF16)          # rope'd k, [dk, s]
        vrow_sb = persist.tile([P, JC, VW], BF16)   # v rows + ones col
        nc.vector.memset(vrow_sb[:, :, DV : DV + 1], 1.0)
        qT = [
            persist.tile([P, SQ], BF16, tag=f"q{h}", name=f"qT{h}")
            for h in range(H)
        ]
        yp = [
            persist.tile([P, SC, 2 * DV], BF16, tag=f"yp{p}", name=f"yp{p}")
            for p in range(4)
        ]
        yT_sb = persist.tile([P, DC, SQ], BF16)

        g = (nc.gpsimd if os.environ.get("USE_GPSIMD", "1") == "1"
             else nc.vector)
        pts = {}

        # ---------------- attention primitives ----------------
        def qk_group(h, jg):
            pq = apq.tile([P, 2, SQ], F32, tag="pq", name=f"pq{h}{jg}")
            dve_bias = jg % 2 == 1
            for c in range(2):
                jc = jg * 2 + c
                if not dve_bias:
                    nc.tensor.matmul(
                        pq[:, c, :], identb[:], biasT_sb[:, jc, :],
                        start=True, stop=False,
                    )
                nc.tensor.matmul(
                    pq[:, c, :],
                    kT_sb[:, jc * P : (jc + 1) * P], qT[h][:],
                    start=dve_bias, stop=True,
                )
            if dve_bias:
                nc.vector.tensor_tensor(
                    pq[:], pq[:], biasT_sb[:, jg * 2 : jg * 2 + 2, :],
                    TT.add,
                )
            t16 = att.tile([P, 2, SQ], F16, tag="t16", name=f"t16_{h}{jg}")
            nc.scalar.activation(
                t16[:], pq[:], AF.Tanh, scale=1.0 / SOFTCAP
            )
            nc.scalar.activation(
                pts[h][:, jg * 2 : jg * 2 + 2, :], t16[:],
                AF.Exp, scale=SOFTCAP,
            )

        # -------- q pipeline (row-direct, two heads per matmul) --------
        def stage_q_proj(hp, t, parts):
            if t == 0:
                parts.append(
                    qpsp.tile([P, SC, 2, DQ], F32, tag="q_ps",
                              name=f"qps{hp}")
                )
            q_ps = parts[0]
            last = DC - 1
            h0 = 2 * hp
            w0 = (h0 % 4) * DQ
            for dc in range(DC):
                nc.tensor.matmul(
                    q_ps[:, t, :, :],
                    xq_sb[:, dc, t * P : (t + 1) * P],
                    wq_sb[:, dc, w0 : w0 + 2 * DQ],
                    start=(dc == 0), stop=(dc == last) and not has_b,
                )
            if has_b:
                nc.tensor.matmul(
                    q_ps[:, t, :, :], ones1[:],
                    brow[:, DKV + h0 * DQ : DKV + (h0 + 2) * DQ],
                    start=False, stop=True,
                )
            st6 = qt.tile([P, 2, 6], F32, tag=f"qst{t}", name=f"qst{hp}_{t}")
            for j in range(2):
                nc.vector.bn_stats(st6[:, j, :], q_ps[:, t, j, :])
            parts.append(st6)

        def stage_q_ln(hp, parts):
            q_ps = parts[0]
            qag = qt.tile([P, SC, 2, 2], F32, tag="qag", name=f"qag{hp}")
            for t in range(SC):
                for j in range(2):
                    nc.vector.bn_aggr(qag[:, t, j, :], parts[1 + t][:, j, :])
            qv = qt.tile([P, SC, 2, 1], F32, tag="qv", name=f"qv{hp}")
            qy = qt.tile([P, SC, 2, 1], F32, tag="qy", name=f"qy{hp}")
            qw2 = qt.tile([P, SC, 2, 1], F32, tag="qw2", name=f"qw2{hp}")
            g.tensor_scalar(qv[:], qag[:, :, :, 1:2], EPS_LN, None, TT.add)
            g.tensor_scalar(qy[:], qv[:], -0.5, 1.5, TT.mult, TT.add)
            for _ in range(3):
                g.tensor_tensor(qw2[:], qy[:], qy[:], TT.mult)
                g.tensor_tensor(qw2[:], qw2[:], qv[:], TT.mult)
                g.tensor_scalar(qw2[:], qw2[:], -0.5, 1.5, TT.mult, TT.add)
                g.tensor_tensor(qy[:], qy[:], qw2[:], TT.mult)
            xnq = qt.tile([P, SC, 2, DQ], BF16, tag="xnq", name=f"xnq{hp}")
            for t in range(SC):
                for j in range(2):
                    nc.vector.tensor_scalar(
                        xnq[:, t, j, :], q_ps[:, t, j, :],
                        qag[:, t, j, 0:1], qy[:, t, j, 0:1],
                        TT.subtract, TT.mult,
                    )
            for j in range(2):
                h = 2 * hp + j
                qz1 = qt.tile([P, SC, HALF], BF16, tag="qz1", name=f"qz1{h}")
                qz2 = qt.tile([P, SC, HALF], BF16, tag="qz2", name=f"qz2{h}")
                qr = qt.tile([P, SC, DQ], BF16, tag="qr", name=f"qr{h}")
                xj = xnq[:, :, j, :]
                g.tensor_tensor(qz1[:], xj[:, :, :HALF], c1q[:], TT.mult)
                g.tensor_tensor(qz2[:], xj[:, :, HALF:], s2nq[:], TT.mult)
                g.tensor_tensor(qr[:, :, :HALF], qz1[:], qz2[:], TT.add)
                g.tensor_tensor(qz1[:], xj[:, :, :HALF], s1q[:], TT.mult)
                g.tensor_tensor(qz2[:], xj[:, :, HALF:], c2q[:], TT.mult)
                g.tensor_tensor(qr[:, :, HALF:], qz1[:], qz2[:], TT.add)
                if has_rbq:
                    g.tensor_tensor(qr[:], qr[:], rbq[:], TT.add)
                for t in range(SC):
                    sct = scr.tile([P, P], BF16, tag="scb", name=f"qbt{h}{t}")
                    nc.tensor.transpose(sct[:], qr[:, t, :], identb[:])
                    nc.vector.tensor_copy(
                        qT[h][:, t * P : (t + 1) * P], sct[:]
                    )

        # =========================================================
        # KV loop with heads 0/1 attention overlapped
        # =========================================================
        for h in (0, 1):
            pts[h] = att.tile([P, JC, SQ], BF16, tag="pt", name=f"pt{h}")

        with (
            tc.tile_pool(name="kvx", bufs=3) as kvx,
            tc.tile_pool(name="kvt", bufs=2) as kvt,
            tc.tile_pool(name="kvrp", bufs=5) as kvrp,
            tc.tile_pool(name="kvps", bufs=1, space="PSUM") as kvps,
        ):
            parts01 = []
            parts1 = []  # pair 1 = heads 2,3

            xts = {}

            def xt_fetch(jc):
                if jc < JC and jc not in xts:
                    xt = kvx.tile([P, DC, P], BF16, tag="xt",
                                  name=f"xt{jc}")
                    nc.sync.dma_start(xt[:], xT[:, jc, :, :])
                    xts[jc] = xt

            def kv_chunk(jc):
                xt_fetch(jc)
                xt_fetch(jc + 1)
                xt_fetch(jc + 2)
                xt = xts.pop(jc)
                if jc == 0:
                    nc.sync.dma_start(xq_sb[:], xq)
                    nc.sync.dma_start(wq_sb[:], wq_t[:, :, 0 : 4 * DQ])
                if jc == 2:
                    nc.sync.dma_start(biasT_sb[:], biasT)
                kv_ps = kvps.tile([P, DKV], F32, tag="kv_ps",
                                  name=f"kvps{jc}")
                last = DC - 1
                for dc in range(DC):
                    nc.tensor.matmul(
                        kv_ps[:], xt[:, dc, :], wkv_sb[:, dc, :],
                        start=(dc == 0), stop=(dc == last) and not has_b,
                    )
                if has_b:
                    nc.tensor.matmul(
                        kv_ps[:], ones1[:], brow[:, 0:DKV],
                        start=False, stop=True,
                    )
                # drain to SBUF immediately (frees the PSUM bank)
                kvr = kvrp.tile([P, DKV], BF16, tag="kvr", name=f"kvr{jc}")
                nc.vector.tensor_copy(kvr[:], kv_ps[:])
                kv_chunk.kvrs[jc % 4] = kvr
                # stats on the bf16 copy; (mean, var) pairs into batch tile
                if jc % 4 == 0:
                    kv_chunk.agg4 = kvt.tile(
                        [P, 4, 2, 2], F32, tag="agg4", name=f"agg4_{jc // 4}"
                    )
                kst = kvt.tile([P, 6], F32, tag="kst", name=f"kst{jc}")
                nc.vector.bn_stats(kst[:], kvr[:, 0:DK])
                nc.vector.bn_aggr(kv_chunk.agg4[:, jc % 4, 0, :], kst[:])
                vst = kvt.tile([P, 6], F32, tag="vst", name=f"vst{jc}")
                nc.vector.bn_stats(vst[:], kvr[:, DK:DKV])
                nc.vector.bn_aggr(kv_chunk.agg4[:, jc % 4, 1, :], vst[:])

            kv_chunk.kvrs = [None] * 4

            def rope_batch(bi):
                # newton rsqrt for the 4 chunks' k/v (no ACT table switch),
                # then normalize and rope on DVE (bf16 4x mode)
                agg4 = kv_chunk.agg4
                nv = kvt.tile([P, 4, 2, 1], F32, tag="nv", name=f"nv{bi}")
                ny = kvt.tile([P, 4, 2, 1], F32, tag="ny", name=f"ny{bi}")
                nw = kvt.tile([P, 4, 2, 1], F32, tag="nw", name=f"nw{bi}")
                V = nc.vector
                V.tensor_scalar(nv[:], agg4[:, :, :, 1:2], EPS_LN, None,
                                TT.add)
                V.tensor_scalar(ny[:], nv[:], -0.5, 1.5, TT.mult, TT.add)
                for _ in range(3):
                    V.tensor_tensor(nw[:], ny[:], ny[:], TT.mult)
                    V.tensor_tensor(nw[:], nw[:], nv[:], TT.mult)
                    V.tensor_scalar(nw[:], nw[:], -0.5, 1.5, TT.mult, TT.add)
                    V.tensor_tensor(ny[:], ny[:], nw[:], TT.mult)
                xnk4 = kvt.tile([P, 4, DK], BF16, tag="xnk4", name=f"xnk4_{bi}")
                for t in range(4):
                    jc = bi * 4 + t
                    kvr = kv_chunk.kvrs[t]
                    nc.vector.tensor_scalar(
                        xnk4[:, t, :], kvr[:, 0:DK],
                        agg4[:, t, 0, 0:1], ny[:, t, 0, 0:1],
                        TT.subtract, TT.mult,
                    )
                    nc.vector.tensor_scalar(
                        vrow_sb[:, jc, 0:DV], kvr[:, DK:DKV],
                        agg4[:, t, 1, 0:1], ny[:, t, 1, 0:1],
                        TT.subtract, TT.mult,
                    )
                jcs = slice(bi * 4, bi * 4 + 4)
                kz1 = kvt.tile([P, 4, HALF], BF16, tag="kz1", name=f"kz1{bi}")
                kz2 = kvt.tile([P, 4, HALF], BF16, tag="kz2", name=f"kz2{bi}")
                kr4 = kvt.tile([P, 4, DK], BF16, tag="kr4", name=f"kr4{bi}")
                V.tensor_tensor(kz1[:], xnk4[:, :, :HALF], c1k[:, jcs, :],
                                TT.mult)
                V.tensor_tensor(kz2[:], xnk4[:, :, HALF:], s2nk[:, jcs, :],
                                TT.mult)
                V.tensor_tensor(kr4[:, :, :HALF], kz1[:], kz2[:], TT.add)
                V.tensor_tensor(kz1[:], xnk4[:, :, :HALF], s1k[:, jcs, :],
                                TT.mult)
                V.tensor_tensor(kz2[:], xnk4[:, :, HALF:], c2k[:, jcs, :],
                                TT.mult)
                V.tensor_tensor(kr4[:, :, HALF:], kz1[:], kz2[:], TT.add)
                if has_rbk:
                    V.tensor_tensor(kr4[:], kr4[:], rbk[:, jcs, :], TT.add)
                for t in range(4):
                    jc = bi * 4 + t
                    scb = scr.tile([P, P], BF16, tag="scb", name=f"kbt{jc}")
                    nc.tensor.transpose(scb[:], kr4[:, t, :], identb[:])
                    nc.vector.tensor_copy(
                        kT_sb[:, jc * P : (jc + 1) * P], scb[:]
                    )

            # schedule: kv chunks, rope batches, stageQ pair0/pair1,
            # heads 0/1 qk groups as kT becomes available
            for jc in range(4):
                kv_chunk(jc)
                if jc >= 1:
                    stage_q_proj(0, jc - 1, parts01)
            stage_q_proj(0, 3, parts01)
            stage_q_ln(0, parts01)
            rope_batch(0)
            for jc in range(4, 8):
                kv_chunk(jc)
                if jc == 5:
                    qk_group(0, 0)
                    qk_group(0, 1)
                if jc == 6:
                    qk_group(1, 0)
                if jc == 7:
                    qk_group(1, 1)
            rope_batch(1)
            for jc in range(8, 12):
                kv_chunk(jc)
                if jc == 8:
                    qk_group(0, 2)
                if jc == 9:
                    qk_group(0, 3)
                    stage_q_proj(1, 0, parts1)
                if jc == 10:
                    qk_group(1, 2)
                    stage_q_proj(1, 1, parts1)
                    wo_sb = persist.tile([P, DC, D], BF16)
                    nc.sync.dma_start(wo_sb[:], wo_t)
                    bor = persist.tile([P, D], F32)
                    nc.sync.dma_start(bor[:], bor_t)
                if jc == 11:
                    qk_group(1, 3)
                    stage_q_proj(1, 2, parts1)
            rope_batch(2)
            for jc in range(12, 16):
                kv_chunk(jc)
                if jc == 12:
                    qk_group(0, 4)
                if jc == 13:
                    qk_group(0, 5)
                    stage_q_proj(1, 3, parts1)
                    nc.sync.dma_start(
                        wq_sb[:], wq_t[:, :, 4 * DQ : 8 * DQ]
                    )
                if jc == 14:
                    qk_group(1, 4)
                if jc == 15:
                    qk_group(1, 5)
                    stage_q_ln(1, parts1)
            rope_batch(3)
            for jg in (6, 7):
                qk_group(0, jg)
            for jg in (6, 7):
                qk_group(1, jg)

        # =========================================================
        # heads 2..7 + PV pipeline
        # =========================================================
        with (
            tc.tile_pool(name="ay", bufs=1, space="PSUM") as ay,
        ):
            def pv_chunk(h, ic):
                y_ps = ay.tile([P, DV + 1], F32, tag="y_ps",
                               name=f"yps{h}{ic}")
                for jc in range(JC):
                    nc.tensor.matmul(
                        y_ps[:],
                        pts[h][:, jc, ic * P : (ic + 1) * P],
                        vrow_sb[:, jc, : DV + 1],
                        start=(jc == 0), stop=(jc == JC - 1),
                    )
                rcp = att.tile([P, 1], F32, tag="rcp", name=f"rcp{h}{ic}")
                nc.vector.reciprocal(rcp[:], y_ps[:, DV : DV + 1])
                nc.vector.tensor_scalar_mul(
                    yp[h // 2][:, ic, (h % 2) * DV : (h % 2 + 1) * DV],
                    y_ps[:, :DV], rcp[:, 0:1],
                )

            def oproj_transposes(p):
                for sc in range(SC):
                    for fcl in range(3):
                        fc = 3 * p + fcl
                        scb = scr.tile([P, P], BF16, tag="scb",
                                       name=f"so{p}{sc}{fcl}")
                        nc.tensor.transpose(
                            scb[:],
                            yp[p][:, sc, fcl * P : (fcl + 1) * P],
                            identb[:],
                        )
                        nc.vector.tensor_copy(
                            yT_sb[:, fc, sc * P : (sc + 1) * P], scb[:]
                        )

            # pv(0) immediately after the overlap section so the pts[0]
            # buffer (reused by pts[2]) frees before head 2's exps
            for ic in range(SC):
                pv_chunk(0, ic)

            parts_by_pair = {}
            for h in range(2, H):
                pts[h] = att.tile([P, JC, SQ], BF16, tag="pt", name=f"pt{h}")
                fillers = []
                hp = h // 2 + 1
                if hp < 4:
                    if h % 2 == 0:
                        partsn = []
                        parts_by_pair[hp] = partsn
                        for t in range(SC):
                            fillers.append(
                                lambda hp=hp, t=t, pn=partsn: stage_q_proj(
                                    hp, t, pn
                                )
                            )
                    else:
                        fillers.append(
                            lambda hp=hp: stage_q_ln(hp, parts_by_pair[hp])
                        )
                # pv of the previous head must fully emit during this head
                # (pts pool has 2 buffers)
                for ic in range(SC):
                    fillers.append(lambda h=h, ic=ic: pv_chunk(h - 1, ic))
                if 4 <= h <= 6:
                    fillers.append(lambda p=h - 4: oproj_transposes(p))

                nf = len(fillers)
                done = 0
                for jg in range(JC // 2):
                    qk_group(h, jg)
                    want = (nf * (jg + 1) + 7) // 8
                    while done < want:
                        fillers[done]()
                        done += 1
            # tail
            for ic in range(SC):
                pv_chunk(H - 1, ic)
            oproj_transposes(3)
            if DEBUG:
                nc.sync.dma_start(dbg_kT[:], kT_sb[:])
                nc.sync.dma_start(dbg_vrow[:], vrow_sb[:])
                nc.sync.dma_start(dbg_qT0[:], qT[0][:])
                nc.sync.dma_start(dbg_pts0[:], pts[0][:])
                nc.sync.dma_start(dbg_yp0[:], yp[0][:])

        apq_cm.__exit__(None, None, None)
        scr_cm.__exit__(None, None, None)
        qps_cm.__exit__(None, None, None)

        # =========================================================
        # Output projection (bf16)
        # =========================================================
        with (
            tc.tile_pool(name="od", bufs=2) as od,
            tc.tile_pool(name="ops", bufs=2, space="PSUM") as ops,
        ):
            for sc in range(SC):
                o_ps = ops.tile([P, D], F32, tag="o_ps", name=f"ops{sc}")
                for fc in range(DC):
                    for n in range(D // 512):
                        nc.tensor.matmul(
                            o_ps[:, n * 512 : (n + 1) * 512],
                            yT_sb[:, fc, sc * P : (sc + 1) * P],
                            wo_sb[:, fc, n * 512 : (n + 1) * 512],
                            start=(fc == 0), stop=(fc == DC - 1),
                        )
                o_sb = od.tile([P, D], F32, tag="o_sb", name=f"osb{sc}")
                nc.vector.tensor_tensor(o_sb[:], o_ps[:], bor[:], TT.add)
                nc.sync.dma_start(out[sc * P : (sc + 1) * P, :], o_sb[:])

    nc.compile()
    return nc


def _host_prep(inputs):
    import ml_dtypes

    f32 = np.float32
    bf16 = ml_dtypes.bfloat16
    x = np.asarray(inputs["x"], f32)
    bias = np.asarray(inputs["attention_bias"], f32)
    g1 = np.asarray(inputs["g1"], f32)
    b1 = np.asarray(inputs["b1"], f32)
    rr1 = np.asarray(inputs["rrms1"], f32)
    Wq = np.asarray(inputs["Wq"], f32)
    Wk = np.asarray(inputs["Wk"], f32)
    Wv = np.asarray(inputs["Wv"], f32)
    qg = np.asarray(inputs["qg"], f32)
    qb = np.asarray(inputs["qb"], f32)
    kg = np.asarray(inputs["kg"], f32)
    kb = np.asarray(inputs["kb"], f32)
    vg = np.asarray(inputs["vg"], f32)
    vb = np.asarray(inputs["vb"], f32)
    Wo = np.asarray(inputs["Wo"], f32)
    bo = np.asarray(inputs["bo"], f32)
    g2 = np.asarray(inputs["g2"], f32)
    b2 = np.asarray(inputs["b2"], f32)
    rr2 = np.asarray(inputs["rrms2"], f32)

    scale1 = (g1 * (1.0 / np.sqrt(rr1 + EPS_RMS))).astype(f32)
    Wq_e = (Wq * scale1[:, None]).astype(f32)
    Wk_e = (Wk * scale1[:, None]).astype(f32)
    Wv_e = (Wv * scale1[:, None]).astype(f32)
    bq_row = (b1 @ Wq).astype(f32)      # [H*DQ]
    bk_row = (b1 @ Wk).astype(f32)      # [DK]
    bv_row = (b1 @ Wv).astype(f32)      # [DV]
    sc_q = f32(DQ) ** f32(-0.5)
    qg_e = (qg * sc_q).astype(f32)
    qb_e = (qb * sc_q).astype(f32)

    scale2 = (g2 * (1.0 / np.sqrt(rr2 + EPS_RMS))).astype(f32)
    vg_rep = np.tile(vg, H)                      # [H*DV]
    Wo_e = (Wo * vg_rep[:, None] * scale2[None, :]).astype(f32)
    vb_fold = (np.tile(vb, H) @ Wo).astype(f32)  # [D]
    bo_e = ((bo + vb_fold) * scale2 + b2).astype(f32)

    freqs = (
        1.0 / (ROPE_BASE ** (np.arange(HALF, dtype=f32) / HALF))
    ).astype(f32)
    ang = np.arange(S, dtype=f32)[:, None] * freqs[None, :]
    cos = np.cos(ang).astype(f32)                        # [S, 64]
    sin = np.sin(ang).astype(f32)

    c1k = (cos * kg[None, :HALF]).astype(bf16)
    s2nk = (-sin * kg[None, HALF:]).astype(bf16)
    s1k = (sin * kg[None, :HALF]).astype(bf16)
    c2k = (cos * kg[None, HALF:]).astype(bf16)

    rbk_f = np.concatenate(
        [cos * kb[None, :HALF] - sin * kb[None, HALF:],
         sin * kb[None, :HALF] + cos * kb[None, HALF:]], axis=1
    ).astype(f32)
    rbq_f = np.concatenate(
        [cos * qb_e[None, :HALF] - sin * qb_e[None, HALF:],
         sin * qb_e[None, :HALF] + cos * qb_e[None, HALF:]], axis=1
    ).astype(f32)
    has_rbk = bool(np.any(rbk_f))
    has_rbq = bool(np.any(rbq_f))
    has_b = bool(np.any(bq_row) or np.any(bk_row) or np.any(bv_row))

    def dev3(a, n):
        return np.ascontiguousarray(
            a.reshape(n, P, a.shape[-1]).transpose(1, 0, 2)
        )

    rep = lambda v: np.ascontiguousarray(
        np.broadcast_to(v[None, :], (P, v.shape[0]))
    )
    wkv = np.concatenate([Wk_e, Wv_e], axis=1)          # [D, DK+DV]
    shared = {
        "c1k": dev3(c1k, JC),
        "s2nk": dev3(s2nk, JC),
        "s1k": dev3(s1k, JC),
        "c2k": dev3(c2k, JC),
        "wq": dev3(Wq_e.astype(bf16), DC),
        "wkv": dev3(wkv.astype(bf16), DC),
        "wo": dev3(Wo_e.astype(bf16), DC),
        "bor": rep(bo_e),
        "identb": np.eye(P, dtype=bf16),
    }
    if has_b:
        shared["brow"] = np.concatenate(
            [bk_row, bv_row, bq_row]
        ).astype(bf16)[None, :]
    if has_rbk:
        shared["rbk"] = dev3(rbk_f.astype(bf16), JC)

    xdev = []
    for b in range(B):
        xTb = np.ascontiguousarray(x[b].T).astype(bf16)      # [D, S]
        xdev.append(np.ascontiguousarray(
            xTb.reshape(DC, P, JC, P).transpose(1, 2, 0, 3)
        ))
    xTs = [np.ascontiguousarray(x[b].T) for b in range(B)]
    in_maps = []
    for c in range(NCORES):
        b = c // 4
        s0 = (c % 4) * SQ
        m = dict(shared)
        m["xT"] = xdev[b]
        m["xq"] = dev3(xTs[b][:, s0 : s0 + SQ].astype(bf16), DC)
        m["biasT"] = dev3(bias[0, 0, s0 : s0 + SQ, :].T.astype(bf16), JC)
        m["c1q"] = dev3(
            (cos[s0 : s0 + SQ] * qg_e[None, :HALF]).astype(bf16), SC
        )
        m["s2nq"] = dev3(
            (-sin[s0 : s0 + SQ] * qg_e[None, HALF:]).astype(bf16), SC
        )
        m["s1q"] = dev3(
            (sin[s0 : s0 + SQ] * qg_e[None, :HALF]).astype(bf16), SC
        )
        m["c2q"] = dev3(
            (cos[s0 : s0 + SQ] * qg_e[None, HALF:]).astype(bf16), SC
        )
        if has_rbq:
            m["rbq"] = dev3(rbq_f[s0 : s0 + SQ].astype(bf16), SC)
        in_maps.append(m)
    return in_maps, has_rbq, has_rbk, has_b


_NC_CACHE = {}


def _get_nc(has_rbq=False, has_rbk=False, has_b=False):
    key = (has_rbq, has_rbk, has_b)
    if key not in _NC_CACHE:
        _NC_CACHE[key] = build_program(has_rbq, has_rbk, has_b)
    return _NC_CACHE[key]


def kernel(**inputs) -> np.ndarray:
    in_maps, has_rbq, has_rbk, has_b = _host_prep(inputs)
    nc = _get_nc(has_rbq, has_rbk, has_b)
    res = bass_utils.run_bass_kernel_spmd(
        nc, in_maps, core_ids=list(range(NCORES))
    )
    outs = res.results
    full = np.empty((B, S, D), np.float32)
    for c in range(NCORES):
        b = c // 4
        s0 = (c % 4) * SQ
        full[b, s0 : s0 + SQ, :] = outs[c]["out"]
    return full


if __name__ == "__main__":
    nc = _get_nc()
    print("build + compile OK")
